# revision 1
# baseline (speedup 1.0000x reference)
"""EnhancedTemporalAttention Trainium2 kernel.

Full module: GroupNorm(32) -> QKV 1x1conv -> 8-head attention (softmax) ->
out 1x1conv + bias -> +residual, on x [4, 512, 2048] fp32.

Sharding: 8 cores = (batch b = core//2) x (query half = core%2).  Each core
computes GroupNorm + K/V projections over the full sequence for its batch
(duplicated across the pair), Q projection + attention + out projection for
its 1024-query half.  Output slices are disjoint; host just concatenates.

All matmuls run as float32r (fp32 storage, reduced-precision multiply at
full PE rate).  Attention uses the transposed-scores layout (keys on
partitions); softmax denominators ride as a 65th ones-row on the V^T
stationary operand; exp runs on ScalarE straight out of PSUM.  The first
head-pair's attention is interleaved with the V^T projection so ScalarE
starts its exp stream early.
"""
import sys

sys.path.insert(0, "/opt/trn_rl_repo")

import numpy as np

import concourse.bacc as bacc
import concourse.bass as bass
import concourse.tile as tile
from concourse import mybir
from concourse.bass_utils import run_bass_kernel_spmd

F32 = mybir.dt.float32
F32R = mybir.dt.float32r

B = 4
C = 512
N = 2048
NQ = 1024          # queries per core
H = 8
D = 64
G = 32             # groupnorm groups
CPG = C // G       # 16 channels per group
EPS = 1e-4
SCALE = D ** -0.5
NT = C // 128      # 4 channel tiles
NKB = N // 128     # 16 key blocks
AF = mybir.ActivationFunctionType
ALU = mybir.AluOpType

# (j, h) chunk sequence in groups of 3 (one exp instruction each).
# First group is head-0 only so a new pair needs just one freed AV
# accumulator slot to start; the rest alternates heads for row-group
# packing.
_h0 = [(j, 0) for j in range(NKB)]
_h1 = [(j, 1) for j in range(NKB)]
CHUNKS = _h0[:3]
_rest0 = _h0[3:]
for _i in range(NKB):
    CHUNKS.append(_h1[_i])
    if _i < len(_rest0):
        CHUNKS.append(_rest0[_i])
GROUPS = [CHUNKS[i:i + 3] for i in range(0, len(CHUNKS), 3)]
assert sorted(CHUNKS) == sorted([(j, h) for j in range(NKB)
                                 for h in range(2)])


def _build(taps=False):
    nc = bacc.Bacc("TRN2", target_bir_lowering=False, debug=False)
    x_in = nc.dram_tensor("x", [C, N], F32, kind="ExternalInput").ap()
    xq_in = nc.dram_tensor("xq", [C, NQ], F32, kind="ExternalInput").ap()
    wqkvT_in = nc.dram_tensor("wqkvT", [C, 3 * C], F32, kind="ExternalInput").ap()
    woutT_in = nc.dram_tensor("woutT", [C, C], F32, kind="ExternalInput").ap()
    gbo_in = nc.dram_tensor("gbo", [C, 3], F32, kind="ExternalInput").ap()
    gblk_in = nc.dram_tensor("gblk", [128, 8], F32, kind="ExternalInput").ap()
    gbt_in = nc.dram_tensor("gbt", [8, 128], F32, kind="ExternalInput").ap()
    y_out = nc.dram_tensor("y", [C, NQ], F32, kind="ExternalOutput").ap()
    # scratch
    kind_t = "ExternalOutput" if taps else "Internal"
    mr_d = nc.dram_tensor("mr_d", [C, 2], F32, kind=kind_t).ap()
    den_d = nc.dram_tensor("den_d", [H * NQ], F32, kind=kind_t).ap()
    den2_d = nc.dram_tensor("den2_d", [H * NQ], F32).ap()
    tap = {}
    if taps:
        for nm, shp in (("t_xn0", [128, N]), ("t_o65a", [128, 512]),
                        ("t_o65b", [128, 512]), ("t_op00", [128, 512])):
            tap[nm] = nc.dram_tensor(nm, shp, F32, kind="ExternalOutput").ap()

    from contextlib import ExitStack
    with tile.TileContext(nc) as tc, ExitStack() as ctx:
        persist = ctx.enter_context(tc.tile_pool(name="persist", bufs=1))
        gn = ctx.enter_context(tc.tile_pool(name="gn", bufs=1))
        pspool = ctx.enter_context(tc.tile_pool(name="ps", bufs=1,
                                                space="PSUM"))
        expp = ctx.enter_context(tc.tile_pool(name="expp", bufs=4))
        o65p = ctx.enter_context(tc.tile_pool(name="o65p", bufs=5))

        # persistent activation tensors
        q_sb = [persist.tile([128, NQ], F32R, tag=f"q{m}", name=f"q{m}")
                for m in range(NT)]
        k_sb = [persist.tile([128, N], F32R, tag=f"k{m}", name=f"k{m}")
                for m in range(NT)]
        vT_sb = [persist.tile([128, H, D + 1], F32R, tag=f"vT{nb}",
                              name=f"vT{nb}") for nb in range(NKB)]

        den_r = den_d.rearrange("(a b) -> a b", b=512)

        def new_S():
            return pspool.tile([128, 3, 512], F32, tag="S", name="S",
                               bufs=2)

        def score_group(qc, m, grp_chunks):
            ng = len(grp_chunks)
            psc = new_S()
            for i, (j, h) in enumerate(grp_chunks):
                nc.tensor.matmul(
                    psc[:, i, :],
                    lhsT=k_sb[m][h * D:(h + 1) * D, j * 128:(j + 1) * 128],
                    rhs=q_sb[m][h * D:(h + 1) * D,
                                qc * 512:(qc + 1) * 512],
                    start=True, stop=True, tile_position=(h * D, 0))
            eT = expp.tile([128, 3, 512], F32R, tag="e", name="e")
            nc.scalar.activation(out=eT[:, 0:ng, :], in_=psc[:, 0:ng, :],
                                 func=AF.Exp, scale=SCALE)
            return eT

        def av_group(qc, m, av, eT, grp_chunks):
            for i, (j, h) in enumerate(grp_chunks):
                nc.tensor.matmul(
                    av[h], lhsT=vT_sb[j][:, 2 * m + h, :], rhs=eT[:, i, :],
                    start=(j == 0), stop=(j == NKB - 1))

        def attn_group(qc, m, av, grp_chunks):
            av_group(qc, m, av, score_group(qc, m, grp_chunks), grp_chunks)

        def pair_drain(qc, m, av):
            o65s = []
            for h in range(2):
                o65 = o65p.tile([128, 512], F32, tag="o65", name="o65")
                nc.vector.tensor_copy(o65[0:D + 1, :], av[h][0:D + 1, :])
                if taps and m == 0 and qc == 0:
                    nc.sync.dma_start(
                        out=tap["t_o65a" if h == 0 else "t_o65b"], in_=o65)
                nc.scalar.dma_start(out=den_r[qc * 8 + m * 2 + h, :],
                                    in_=o65[D:D + 1, :])
                o65s.append(o65)
            return o65s

        with tc.tile_pool(name="xpool", bufs=1) as xpool, \
             tc.tile_pool(name="xnpool", bufs=1) as xnpool, \
             tc.tile_pool(name="wq", bufs=1) as wqp:
            # ---- input loads, critical-path first: x (in 512-col chunks
            # so bn_stats pipelines), then qkv weights ----
            X = []    # f32r storage; XF = f32 views for DVE reads
            for t in range(NT):
                xt = xpool.tile([128, N], F32R, tag=f"X{t}", name=f"X{t}")
                for sg in range(4):
                    nc.sync.dma_start(
                        out=xt[:, sg * 512:(sg + 1) * 512],
                        in_=x_in[t * 128:(t + 1) * 128,
                                 sg * 512:(sg + 1) * 512].bitcast(F32R))
                X.append(xt)
            XF = [xt.bitcast(F32) for xt in X]
            # ACT table preload off the critical path (Sqrt now; Exp is
            # chained after the real Sqrt below so it can't evict it early)
            eps_t = gn.tile([G, 1], F32, tag="eps_t")
            nc.vector.memset(eps_t, EPS)
            sqw = gn.tile([G, 1], F32, tag="sqw")
            nc.scalar.activation(out=sqw, in_=eps_t, func=AF.Sqrt)
            gblk = gn.tile([128, 8], F32R, tag="gblk")
            nc.sync.dma_start(out=gblk, in_=gblk_in.bitcast(F32R))
            gbt = gn.tile([8, 128], F32R, tag="gbt")
            nc.sync.dma_start(out=gbt, in_=gbt_in.bitcast(F32R))
            XQ = []
            gbo = []
            for t in range(NT):
                xqt = persist.tile([128, NQ], F32, tag=f"XQ{t}",
                                   name=f"XQ{t}")
                nc.sync.dma_start(out=xqt,
                                  in_=xq_in[t * 128:(t + 1) * 128, :])
                XQ.append(xqt)
                gt = persist.tile([128, 3], F32, tag=f"gbo{t}",
                                  name=f"gbo{t}")
                nc.sync.dma_start(out=gt,
                                  in_=gbo_in[t * 128:(t + 1) * 128, :])
                gbo.append(gt)
            wT = [wqp.tile([128, 3 * C], F32R, tag=f"wT{kc}",
                           name=f"wT{kc}") for kc in range(NT)]
            for sl in (1, 0, 2):   # k first (k-proj is emitted first)
                for kc in range(NT):
                    nc.sync.dma_start(
                        out=wT[kc][:, sl * C:(sl + 1) * C],
                        in_=wqkvT_in[kc * 128:(kc + 1) * 128,
                                     sl * C:(sl + 1) * C].bitcast(F32R))

            # ---- GroupNorm stats: bn_stats -> per-channel (mean, E[x^2])
            # -> PE block-ones matmul reduces 16-channel groups ----
            mvv = []
            for t in range(NT):
                stats = gn.tile([128, 4, 6], F32, tag=f"st{t}",
                                name=f"st{t}")
                for sg in range(4):
                    nc.vector.bn_stats(out=stats[:, sg, :],
                                       in_=XF[t][:, sg * 512:(sg + 1) * 512])
                mv = gn.tile([128, 2], F32, tag=f"mv{t}", name=f"mv{t}")
                nc.vector.bn_aggr(out=mv, in_=stats)
                mt = gn.tile([128, 2], F32R, tag=f"mvv{t}", name=f"mvv{t}")
                nc.vector.tensor_copy(mt[:, 0:1], mv[:, 0:1])
                sqm = gn.tile([128, 1], F32, tag=f"sqm{t}", name=f"sqm{t}")
                # E[x^2] = var + mean^2
                nc.vector.tensor_mul(sqm, mv[:, 0:1], mv[:, 0:1])
                nc.vector.tensor_tensor(out=mt[:, 1:2], in0=mv[:, 1:2],
                                        in1=sqm, op=ALU.add)
                mvv.append(mt)
            g8ps = new_S()     # group sums land in psum bank 0, [8, 8]
            for t in range(NT):
                nc.tensor.matmul(g8ps[0:8, 0, t * 2:(t + 1) * 2],
                                 lhsT=gblk, rhs=mvv[t],
                                 start=(t == 0), stop=(t == NT - 1),
                                 skip_group_check=True)
            g8 = gn.tile([8, NT, 2], F32, tag="g8")
            nc.vector.tensor_copy(g8.rearrange("p t s -> p (t s)"),
                                  g8ps[0:8, 0, 0:8])
            mean8 = gn.tile([8, NT], F32, tag="mean8")
            nc.vector.tensor_scalar_mul(mean8, g8[:, :, 0], 1.0 / CPG)
            ex28 = gn.tile([8, NT], F32, tag="ex28")
            nc.vector.tensor_scalar_mul(ex28, g8[:, :, 1], 1.0 / CPG)
            msq8 = gn.tile([8, NT], F32, tag="msq8")
            nc.vector.tensor_mul(msq8, mean8, mean8)
            var8 = gn.tile([8, NT], F32, tag="var8")
            nc.vector.tensor_tensor(out=var8, in0=ex28, in1=msq8,
                                    op=ALU.subtract)
            std8 = gn.tile([8, NT], F32, tag="std8")
            nc.scalar.activation(out=std8, in_=var8, func=AF.Sqrt,
                                 bias=eps_t[0:8, :])
            rstd8 = gn.tile([8, NT], F32, tag="rstd8")
            nc.vector.reciprocal(rstd8, std8)
            # preload the Exp table now; input std8 forces it after Sqrt
            warm = gn.tile([8, NT], F32, tag="warm")
            nc.scalar.activation(out=warm, in_=std8, func=AF.Exp)
            mr8 = gn.tile([8, NT, 2], F32R, tag="mr8")
            nc.vector.tensor_copy(mr8[:, :, 0:1],
                                  mean8.rearrange("p (t o) -> p t o", o=1))
            nc.vector.tensor_copy(mr8[:, :, 1:2],
                                  rstd8.rearrange("p (t o) -> p t o", o=1))
            # broadcast group stats to channels via a K=8 ones matmul
            msps = new_S()
            for t in range(NT):
                nc.tensor.matmul(msps[:, 0, t * 2:(t + 1) * 2],
                                 lhsT=gbt, rhs=mr8[:, t, :],
                                 start=(t == 0), stop=(t == NT - 1),
                                 skip_group_check=True)
            mscall = msps[:, 0, 0:2 * NT].rearrange("p (t s) -> p t s", s=2)

            # per-channel scale/bias, then normalize (in place over X)
            xnq = []
            for t in range(NT):
                scale_c = gn.tile([128, 1], F32, tag=f"sc{t}", name=f"sc{t}")
                nc.vector.tensor_mul(scale_c, mscall[:, t, 1:2],
                                     gbo[t][:, 0:1])
                tmp = gn.tile([128, 1], F32, tag=f"tmp{t}", name=f"tmp{t}")
                nc.vector.tensor_mul(tmp, mscall[:, t, 0:1], scale_c)
                bias_c = gn.tile([128, 1], F32, tag=f"bc{t}", name=f"bc{t}")
                nc.vector.tensor_tensor(out=bias_c, in0=gbo[t][:, 1:2],
                                        in1=tmp, op=ALU.subtract)
                xnt = X[t]
                nc.vector.tensor_scalar(out=xnt, in0=XF[t], scalar1=scale_c,
                                        scalar2=bias_c, op0=ALU.mult,
                                        op1=ALU.add)
                if taps and t == 0:
                    nc.sync.dma_start(out=tap["t_xn0"], in_=xnt.bitcast(F32))
                xnqt = xnpool.tile([128, NQ], F32R, tag=f"xnq{t}",
                                   name=f"xnq{t}")
                nc.vector.tensor_scalar(out=xnqt, in0=XQ[t], scalar1=scale_c,
                                        scalar2=bias_c, op0=ALU.mult,
                                        op1=ALU.add)
                xnq.append(xnqt)
            xn = X

            # ---------- QKV projections ----------
            ones_sb = gn.tile([128, H], F32, tag="ones_sb")
            nc.vector.memset(ones_sb, 1.0)
            def kproj(m, ncx):
                psy = new_S()
                ps = psy[:, 0, :]
                for kc in range(NT):
                    nc.tensor.matmul(
                        ps, lhsT=wT[kc][:, C + m * 128:C + (m + 1) * 128],
                        rhs=xn[kc][:, ncx * 512:(ncx + 1) * 512],
                        start=(kc == 0), stop=(kc == NT - 1))
                nc.vector.tensor_copy(
                    k_sb[m][:, ncx * 512:(ncx + 1) * 512], ps)

            def qproj(m, ncx):
                psy = new_S()
                ps = psy[:, 0, :]
                for kc in range(NT):
                    nc.tensor.matmul(
                        ps, lhsT=wT[kc][:, m * 128:(m + 1) * 128],
                        rhs=xnq[kc][:, ncx * 512:(ncx + 1) * 512],
                        start=(kc == 0), stop=(kc == NT - 1))
                nc.vector.tensor_copy(
                    q_sb[m][:, ncx * 512:(ncx + 1) * 512], ps)

            # m=0 projections first so pair (0,0) scores can interleave
            for ncx in range(N // 512):
                kproj(0, ncx)
            for ncx in range(NQ // 512):
                qproj(0, ncx)
            rest = ([(kproj, m, ncx) for m in range(1, NT)
                     for ncx in range(N // 512)]
                    + [(qproj, m, ncx) for m in range(1, NT)
                       for ncx in range(NQ // 512)])
            # interleave early scores (bounded by eT buffering) with the
            # remaining projections
            eTs = {}
            gi_next = 0
            for idx, (fn, m, ncx) in enumerate(rest):
                fn(m, ncx)
                if idx % 2 == 1 and gi_next < 4:
                    eTs[gi_next] = score_group(0, 0, GROUPS[gi_next])
                    gi_next += 1
            # v^T production; AV groups follow their key blocks, scores for
            # later groups emitted just-in-time
            av00 = [pspool.tile([D + 1, 512], F32, tag="av", name="av",
                                bufs=2) for _ in range(2)]
            gi_by_maxj = {}
            for gi, gch in enumerate(GROUPS):
                mj = max(j for j, _ in gch)
                gi_by_maxj.setdefault(mj, []).append(gi)
            for nb in range(NKB):
                psy = new_S()
                ps = psy[:, 0, :]
                for kc in range(NT):
                    nc.tensor.matmul(
                        ps, lhsT=xn[kc][:, nb * 128:(nb + 1) * 128],
                        rhs=wT[kc][:, 2 * C:3 * C],
                        start=(kc == 0), stop=(kc == NT - 1))
                nc.vector.tensor_copy(
                    vT_sb[nb][:, :, 0:D],
                    ps.rearrange("p (h d) -> p h d", h=H))
                nc.vector.tensor_copy(
                    vT_sb[nb][:, :, D:D + 1],
                    ones_sb.rearrange("p (h o) -> p h o", o=1))
                for gi in gi_by_maxj.get(nb, []):
                    if gi in eTs:
                        av_group(0, 0, av00, eTs.pop(gi), GROUPS[gi])
                    else:
                        attn_group(0, 0, av00, GROUPS[gi])
            o65s00 = pair_drain(0, 0, av00)

        # ---------- rest of attention ----------
        with tc.tile_pool(name="opp", bufs=1) as opp, \
             tc.tile_pool(name="ytp", bufs=4) as ytp, \
             tc.tile_pool(name="rbp", bufs=8) as rbp:
            opair = {(kc, nck): opp.tile([128, 512], F32R,
                                         tag=f"op{kc}_{nck}",
                                         name=f"op{kc}_{nck}")
                     for kc in range(NT) for nck in range(2)}
            woutT = []
            for t in range(NT):
                wt = opp.tile([128, C], F32R, tag=f"woT{t}", name=f"woT{t}")
                nc.sync.dma_start(
                    out=wt,
                    in_=woutT_in[t * 128:(t + 1) * 128, :].bitcast(F32R))
                woutT.append(wt)

            def do_pair(qc, m):
                av = [pspool.tile([D + 1, 512], F32, tag="av", name="av",
                                  bufs=2) for _ in range(2)]
                for gch in GROUPS:
                    attn_group(qc, m, av, gch)
                return pair_drain(qc, m, av)

            def do_norm(qc, m, o65s):
                """reciprocal of the pair's dens (repacked [128,8]),
                broadcast back, normalize into opair[(m, qc)]."""
                r0 = qc * 8 + m * 2
                dpack = rbp.tile([128, 8], F32, tag="dpack", name="dpack")
                nc.sync.dma_start(
                    out=dpack,
                    in_=den_d.rearrange("(a b) -> a b", b=8)[
                        r0 * 64:(r0 + 2) * 64, :])
                nc.vector.reciprocal(dpack, dpack)
                nc.scalar.dma_start(
                    out=den2_d.rearrange("(a b) -> a b", b=8)[
                        r0 * 64:(r0 + 2) * 64, :],
                    in_=dpack)
                op = opair[(m, qc)]
                # head 0: rb at partitions 0..63, multiply into opair rows
                rb = rbp.tile([D, 512], F32, tag="rb", name="rb")
                nc.sync.dma_start(
                    out=rb,
                    in_=bass.AP(tensor=den2_d.tensor,
                                offset=den2_d.offset + r0 * 512,
                                ap=[[0, D], [1, 512]]))
                nc.vector.tensor_mul(op[0:D, :], o65s[0][0:D, :], rb)
                # head 1: shift raw rows into opair[64:128] while loading
                # its reciprocal bcast at partitions 64..127, then multiply
                # in place
                rb2 = rbp.tile([128, 512], F32, tag="rb2", name="rb2")
                nc.sync.dma_start(
                    out=rb2[D:2 * D, :],
                    in_=bass.AP(tensor=den2_d.tensor,
                                offset=den2_d.offset + (r0 + 1) * 512,
                                ap=[[0, D], [1, 512]]))
                nc.scalar.dma_start(out=op[D:2 * D, :],
                                    in_=o65s[1][0:D, :].bitcast(F32R))
                nc.vector.tensor_mul(op[D:2 * D, :],
                                     op[D:2 * D, :].bitcast(F32),
                                     rb2[D:2 * D, :])

            def op_start(qc, m2, nkc):
                psy = new_S()
                ps = psy[:, 0, :]
                for kc in range(nkc):
                    nc.tensor.matmul(
                        ps, lhsT=woutT[kc][:, m2 * 128:(m2 + 1) * 128],
                        rhs=opair[(kc, qc)],
                        start=(kc == 0), stop=False)
                return ps

            def op_finish(qc, m2, ps, kc0):
                for kc in range(kc0, NT):
                    nc.tensor.matmul(
                        ps, lhsT=woutT[kc][:, m2 * 128:(m2 + 1) * 128],
                        rhs=opair[(kc, qc)],
                        start=False, stop=(kc == NT - 1))
                yt = ytp.tile([128, 512], F32, tag="yt", name="yt")
                nc.vector.scalar_tensor_tensor(
                    out=yt, in0=ps, scalar=gbo[m2][:, 2:3],
                    in1=XQ[m2][:, qc * 512:(qc + 1) * 512],
                    op0=ALU.add, op1=ALU.add)
                nc.sync.dma_start(
                    out=y_out[m2 * 128:(m2 + 1) * 128,
                              qc * 512:(qc + 1) * 512],
                    in_=yt)

            def do_outproj(qc):
                for m2 in range(NT):
                    op_finish(qc, m2, op_start(qc, m2, 1), 1)

            # software-pipelined emission: out-proj of half 0 lands in the
            # middle of half 1's attention stream
            do_norm(0, 0, o65s00)
            for m in range(1, NT):
                do_norm(0, m, do_pair(0, m))
            for m in range(2):
                do_norm(1, m, do_pair(1, m))
            do_outproj(0)
            if taps:
                nc.sync.dma_start(out=tap["t_op00"],
                                  in_=opair[(0, 0)].bitcast(F32))
            do_norm(1, 2, do_pair(1, 2))
            o65s13 = do_pair(1, 3)
            # kc=0..2 partial sums for two output blocks run while the
            # (1,3) den chain drains; kc=3 lands after the last normalize
            ps0 = op_start(1, 0, NT - 1)
            ps1 = op_start(1, 1, NT - 1)
            do_norm(1, 3, o65s13)
            op_finish(1, 0, ps0, NT - 1)
            op_finish(1, 1, ps1, NT - 1)
            for m2 in (2, 3):
                op_finish(1, m2, op_start(1, m2, 1), 1)

    nc.compile()
    return nc


_NC = None


def _get_nc():
    global _NC
    if _NC is None:
        _NC = _build()
    return _NC


def _gblk():
    g = np.zeros((128, 8), dtype=np.float32)
    for p in range(128):
        g[p, p // CPG] = 1.0
    return g


def kernel(x, gn_gamma, gn_beta, w_qkv, w_out, b_out, trace=False):
    x = np.ascontiguousarray(np.asarray(x, dtype=np.float32))
    wqkvT = np.ascontiguousarray(np.asarray(w_qkv, np.float32).T)
    woutT = np.ascontiguousarray(np.asarray(w_out, np.float32).T)
    gbo = np.ascontiguousarray(np.stack(
        [np.asarray(gn_gamma, np.float32).reshape(C),
         np.asarray(gn_beta, np.float32).reshape(C),
         np.asarray(b_out, np.float32).reshape(C)], axis=1))
    gblk = _gblk()
    gbt = np.ascontiguousarray(gblk.T)

    nc = _get_nc()
    in_maps = []
    for core in range(8):
        b, half = core // 2, core % 2
        in_maps.append({
            "x": x[b],
            "xq": np.ascontiguousarray(x[b][:, half * NQ:(half + 1) * NQ]),
            "wqkvT": wqkvT,
            "woutT": woutT,
            "gbo": gbo,
            "gblk": gblk,
            "gbt": gbt,
        })
    res = run_bass_kernel_spmd(nc, in_maps, core_ids=list(range(8)),
                               trace=trace)
    y = np.empty((B, C, N), dtype=np.float32)
    for core in range(8):
        b, half = core // 2, core % 2
        y[b][:, half * NQ:(half + 1) * NQ] = res.results[core]["y"]
    if trace:
        kernel.last_results = res
    return y



# revision 5
# speedup vs baseline: 1.2609x; 1.2609x over previous
"""EnhancedTemporalAttention Trainium2 kernel (v2).

Full module: GroupNorm(32) -> QKV 1x1conv -> 8-head attention (softmax) ->
out 1x1conv + bias -> +residual, on x [4, 512, 2048] fp32.

Sharding: 8 cores = (batch b = core//2) x (head-half hg = core%2).  Each
core computes GroupNorm stats + its 4 heads' Q/K/V over the full sequence,
attention for all 2048 queries, and a partial out-projection (contraction
over its 256 channels).  Host sums the two partials per batch and adds
residual + b_out exactly in fp32.

GroupNorm is folded into the QKV weights: w' = w * scale_c (per input
channel, scaled in place on Pool), plus a K=1 matvec for the bias term
which rides the projection PSUM->SBUF copies as a per-partition bias.

Attention uses transposed scores (keys on partitions, [k,q] layout); exp
splits between ACT (exact, 19/32 per pair) and DVE (Schraudolph int16
bit-trick into bf16 bits, 13/32).  AV runs with eT stationary / vT moving
so each matmul is only 65 output rows; softmax denominators ride a ones
column on vT; normalization is a per-partition multiply in [q,d] layout,
then a PE transpose (bf16) back to [c,q] for the out-projection, whose
PSUM result DMAs straight to DRAM.
"""
import sys

sys.path.insert(0, "/opt/trn_rl_repo")

import numpy as np
import ml_dtypes

import concourse.bacc as bacc
import concourse.bass as bass
import concourse.tile as tile
from concourse import mybir
from concourse.bass_utils import run_bass_kernel_spmd

F32 = mybir.dt.float32
F32R = mybir.dt.float32r
BF16 = mybir.dt.bfloat16
I16 = mybir.dt.int16

B = 4
C = 512
N = 2048
H = 8
HL = 4             # local heads per core
D = 64
G = 32             # groupnorm groups
CPG = C // G       # 16 channels per group
EPS = 1e-4
SCALE = D ** -0.5
NT = C // 128      # 4 input-channel tiles
MT = 2             # local qkv channel tiles (256 local channels)
NKB = N // 128     # 16 key blocks
QC = 4             # query chunks of 512
AF = mybir.ActivationFunctionType
ALU = mybir.AluOpType

# Schraudolph exp into bf16 bits: i16 = s*A_S + B_S, bitcast -> bf16
A_S = 184.6650085 * SCALE
B_S = 16249.1
ACT_H1 = (0, 5, 10)   # j's whose h1 exp goes to ACT (exact) not DVE


def _build(taps=False):
    nc = bacc.Bacc("TRN2", target_bir_lowering=False, debug=False)
    x_in = nc.dram_tensor("x", [C, N], BF16, kind="ExternalInput").ap()
    wqkvT_in = nc.dram_tensor("wqkvT", [C, 6 * 128], BF16,
                              kind="ExternalInput").ap()
    woutT_in = nc.dram_tensor("woutT", [MT * 128, C], BF16,
                              kind="ExternalInput").ap()
    gbo_in = nc.dram_tensor("gbo", [C, 2], F32, kind="ExternalInput").ap()
    gblk_in = nc.dram_tensor("gblk", [128, 8], F32, kind="ExternalInput").ap()
    gbt_in = nc.dram_tensor("gbt", [8, 128], F32, kind="ExternalInput").ap()
    id_in = nc.dram_tensor("ident", [128, 128], BF16,
                           kind="ExternalInput").ap()
    y_out = nc.dram_tensor("y", [C, N], F32, kind="ExternalOutput").ap()

    from contextlib import ExitStack
    with tile.TileContext(nc) as tc, ExitStack() as ctx:
        persist = ctx.enter_context(tc.tile_pool(name="persist", bufs=1))
        gn = ctx.enter_context(tc.tile_pool(name="gn", bufs=1))
        pspool = ctx.enter_context(tc.tile_pool(name="ps", bufs=1,
                                                space="PSUM"))
        expp = ctx.enter_context(tc.tile_pool(name="expp", bufs=1))
        drp = ctx.enter_context(tc.tile_pool(name="drp", bufs=1))

        # ---- persistent tiles ----
        X = [persist.tile([128, N], BF16, tag=f"X{t}", name=f"X{t}")
             for t in range(NT)]
        wT = [persist.tile([128, 6 * 128], BF16, tag=f"wT{kc}",
                           name=f"wT{kc}") for kc in range(NT)]
        woutT = [persist.tile([128, C], BF16, tag=f"woT{m}", name=f"woT{m}")
                 for m in range(MT)]
        q_sb = [persist.tile([128, N], BF16, tag=f"q{m}", name=f"q{m}")
                for m in range(MT)]
        k_sb = [persist.tile([128, N], BF16, tag=f"k{m}", name=f"k{m}")
                for m in range(MT)]
        vT_sb = [persist.tile([128, HL, 66], BF16, tag=f"vT{nb}",
                              name=f"vT{nb}") for nb in range(NKB)]
        ident = persist.tile([128, 128], BF16, tag="ident", name="ident")
        nc.sync.dma_start(out=ident, in_=id_in)

        # PSUM rings: S 3x2KB + av 4KB + ops 3x2KB = 16KB exactly
        def new_S():
            return pspool.tile([128, 512], F32, tag="S", name="S", bufs=3)

        def new_ops():
            return pspool.tile([128, 512], F32, tag="ops", name="ops",
                               bufs=3)

        def new_av():
            return pspool.tile([128, 8, 128], F32, tag="av", name="av",
                               bufs=1)

        # ---- input loads: x in 512-col chunks (bn_stats pipelines) ----
        for t in range(NT):
            for sg in range(4):
                nc.sync.dma_start(
                    out=X[t][:, sg * 512:(sg + 1) * 512],
                    in_=x_in[t * 128:(t + 1) * 128, sg * 512:(sg + 1) * 512])
        for kc in range(NT):
            nc.sync.dma_start(out=wT[kc],
                              in_=wqkvT_in[kc * 128:(kc + 1) * 128, :])
        for m in range(MT):
            nc.sync.dma_start(out=woutT[m],
                              in_=woutT_in[m * 128:(m + 1) * 128, :])
        gblk = gn.tile([128, 8], F32R, tag="gblk")
        nc.sync.dma_start(out=gblk, in_=gblk_in.bitcast(F32R))
        gbt = gn.tile([8, 128], F32R, tag="gbt")
        nc.sync.dma_start(out=gbt, in_=gbt_in.bitcast(F32R))
        gbo = []
        for t in range(NT):
            gt = gn.tile([128, 2], F32, tag=f"gbo{t}", name=f"gbo{t}")
            nc.sync.dma_start(out=gt, in_=gbo_in[t * 128:(t + 1) * 128, :])
            gbo.append(gt)

        # ---- GroupNorm stats ----
        eps_t = gn.tile([G, 1], F32, tag="eps_t")
        nc.vector.memset(eps_t, EPS)
        sqw = gn.tile([G, 1], F32, tag="sqw")
        nc.scalar.activation(out=sqw, in_=eps_t, func=AF.Sqrt)
        mvv = []
        for t in range(NT):
            stats = gn.tile([128, 4, 6], F32, tag=f"st{t}", name=f"st{t}")
            for sg in range(4):
                nc.vector.bn_stats(out=stats[:, sg, :],
                                   in_=X[t][:, sg * 512:(sg + 1) * 512])
            mv = gn.tile([128, 2], F32, tag=f"mv{t}", name=f"mv{t}")
            nc.vector.bn_aggr(out=mv, in_=stats)
            mt = gn.tile([128, 2], F32R, tag=f"mvv{t}", name=f"mvv{t}")
            nc.vector.tensor_copy(mt[:, 0:1], mv[:, 0:1])
            sqm = gn.tile([128, 1], F32, tag=f"sqm{t}", name=f"sqm{t}")
            nc.vector.tensor_mul(sqm, mv[:, 0:1], mv[:, 0:1])
            nc.vector.tensor_tensor(out=mt[:, 1:2], in0=mv[:, 1:2],
                                    in1=sqm, op=ALU.add)
            mvv.append(mt)
        g8ps = new_ops()
        for t in range(NT):
            nc.tensor.matmul(g8ps[0:8, t * 2:(t + 1) * 2],
                             lhsT=gblk, rhs=mvv[t],
                             start=(t == 0), stop=(t == NT - 1),
                             skip_group_check=True)
        g8 = gn.tile([8, NT, 2], F32, tag="g8")
        nc.vector.tensor_copy(g8.rearrange("p t s -> p (t s)"),
                              g8ps[0:8, 0:8])
        mean8 = gn.tile([8, NT], F32, tag="mean8")
        nc.vector.tensor_scalar_mul(mean8, g8[:, :, 0], 1.0 / CPG)
        ex28 = gn.tile([8, NT], F32, tag="ex28")
        nc.vector.tensor_scalar_mul(ex28, g8[:, :, 1], 1.0 / CPG)
        msq8 = gn.tile([8, NT], F32, tag="msq8")
        nc.vector.tensor_mul(msq8, mean8, mean8)
        var8 = gn.tile([8, NT], F32, tag="var8")
        nc.vector.tensor_tensor(out=var8, in0=ex28, in1=msq8,
                                op=ALU.subtract)
        std8 = gn.tile([8, NT], F32, tag="std8")
        nc.scalar.activation(out=std8, in_=var8, func=AF.Sqrt,
                             bias=eps_t[0:8, :])
        rstd8 = gn.tile([8, NT], F32, tag="rstd8")
        nc.vector.reciprocal(rstd8, std8)
        # preload the Exp table; chained after the real Sqrt via std8
        warm = gn.tile([8, NT], F32, tag="warm")
        nc.scalar.activation(out=warm, in_=std8, func=AF.Exp)
        mr8 = gn.tile([8, NT, 2], F32R, tag="mr8")
        nc.vector.tensor_copy(mr8[:, :, 0:1],
                              mean8.rearrange("p (t o) -> p t o", o=1))
        nc.vector.tensor_copy(mr8[:, :, 1:2],
                              rstd8.rearrange("p (t o) -> p t o", o=1))
        msps = new_ops()
        for t in range(NT):
            nc.tensor.matmul(msps[:, t * 2:(t + 1) * 2],
                             lhsT=gbt, rhs=mr8[:, t, :],
                             start=(t == 0), stop=(t == NT - 1),
                             skip_group_check=True)
        mscall = msps[:, 0:2 * NT].rearrange("p (t s) -> p t s", s=2)

        # per-channel scale_c = rstd*gamma, bias_c = beta - mean*scale_c
        qkvb_ps = None
        scale_c = []
        for t in range(NT):
            sc = gn.tile([128, 1], F32, tag=f"sc{t}", name=f"sc{t}")
            nc.vector.tensor_mul(sc, mscall[:, t, 1:2], gbo[t][:, 0:1])
            scale_c.append(sc)
            tmp = gn.tile([128, 1], F32, tag=f"tmp{t}", name=f"tmp{t}")
            nc.vector.tensor_mul(tmp, mscall[:, t, 0:1], sc)
            bias_c = gn.tile([128, 1], F32, tag=f"bc{t}", name=f"bc{t}")
            nc.vector.tensor_tensor(out=bias_c, in0=gbo[t][:, 1:2],
                                    in1=tmp, op=ALU.subtract)
            bb = gn.tile([128, 1], BF16, tag=f"bb{t}", name=f"bb{t}")
            nc.vector.tensor_copy(bb, bias_c)
            # qkv bias matvec against RAW weights (before scaling)
            if qkvb_ps is None:
                qkvb_ps = new_ops()
            for oc in range(6):
                nc.tensor.matmul(qkvb_ps[:, 6 * t + oc:6 * t + oc + 1],
                                 lhsT=wT[t][:, oc * 128:(oc + 1) * 128],
                                 rhs=bb, start=True, stop=True,
                                 skip_group_check=True)
        # reduce the 4 kc partials: qkvb[:, oc] = sum_t qkvb_ps[:, 6t+oc]
        qkvb_all = gn.tile([128, NT, 6], F32, tag="qkvb_all")
        nc.vector.tensor_copy(qkvb_all.rearrange("p t s -> p (t s)"),
                              qkvb_ps[:, 0:24])
        qkvb01 = gn.tile([128, 6], F32, tag="qkvb01")
        nc.vector.tensor_tensor(out=qkvb01, in0=qkvb_all[:, 0, :],
                                in1=qkvb_all[:, 1, :], op=ALU.add)
        qkvb23 = gn.tile([128, 6], F32, tag="qkvb23")
        nc.vector.tensor_tensor(out=qkvb23, in0=qkvb_all[:, 2, :],
                                in1=qkvb_all[:, 3, :], op=ALU.add)
        qkvb = gn.tile([128, 6], F32, tag="qkvb")
        nc.vector.tensor_tensor(out=qkvb, in0=qkvb01, in1=qkvb23,
                                op=ALU.add)
        # scale weights in place (per input-channel partition) on Pool
        for t in range(NT):
            nc.gpsimd.tensor_scalar(out=wT[t], in0=wT[t],
                                    scalar1=scale_c[t], scalar2=None,
                                    op0=ALU.mult)

        # ---- projections ----
        def kq_proj(which, m, ncx, eng):
            ps = new_ops()
            col0 = (m if which == "q" else 2 + m) * 128
            for kc in range(NT):
                nc.tensor.matmul(
                    ps, lhsT=wT[kc][:, col0:col0 + 128],
                    rhs=X[kc][:, ncx * 512:(ncx + 1) * 512],
                    start=(kc == 0), stop=(kc == NT - 1))
            dst = (q_sb if which == "q" else k_sb)[m][
                :, ncx * 512:(ncx + 1) * 512]
            boff = (0 if which == "q" else 2) + m
            if eng == "act":
                nc.scalar.activation(out=dst, in_=ps, func=AF.Identity,
                                     bias=qkvb[:, boff:boff + 1])
            else:
                nc.vector.tensor_scalar(out=dst, in0=ps,
                                        scalar1=qkvb[:, boff:boff + 1],
                                        scalar2=None, op0=ALU.add)

        def v_proj(nb, eng):
            ps = new_ops()
            for kc in range(NT):
                nc.tensor.matmul(
                    ps[:, 0:256], lhsT=X[kc][:, nb * 128:(nb + 1) * 128],
                    rhs=wT[kc][:, 4 * 128:6 * 128],
                    start=(kc == 0), stop=(kc == NT - 1))
            src = ps[:, 0:256].rearrange("p (h d) -> p h d", h=HL)
            dst = vT_sb[nb][:, :, 0:64]
            # v bias is folded at the opair copy; here plain convert
            if eng == "act":
                nc.scalar.activation(out=dst, in_=src, func=AF.Copy)
            else:
                nc.vector.tensor_copy(dst, src)
            nc.gpsimd.memset(vT_sb[nb][:, :, 64:65], 1.0)

        engs = ["act", "dve"]
        for m in range(MT):
            for ncx in range(4):
                kq_proj("k", m, ncx, engs[(m * 4 + ncx) % 2])
        for m in range(MT):
            for ncx in range(4):
                kq_proj("q", m, ncx, engs[(m * 4 + ncx + 1) % 2])
        for nb in range(NKB):
            v_proj(nb, engs[nb % 2])

        # ---- attention ----
        pairs = [(qc, m) for qc in range(QC) for m in range(MT)]
        opair = {}

        def emit_scores(qc, m, j):
            s0 = new_S()
            nc.tensor.matmul(
                s0, lhsT=k_sb[m][0:64, j * 128:(j + 1) * 128],
                rhs=q_sb[m][0:64, qc * 512:(qc + 1) * 512],
                start=True, stop=True, tile_position=(0, 0))
            s1 = new_S()
            nc.tensor.matmul(
                s1, lhsT=k_sb[m][64:128, j * 128:(j + 1) * 128],
                rhs=q_sb[m][64:128, qc * 512:(qc + 1) * 512],
                start=True, stop=True, tile_position=(64, 0))
            return s0, s1

        def emit_exp(sS, eT, j):
            s0, s1 = sS
            eb = eT.bitcast(BF16)
            nc.scalar.activation(out=eb[:, 0:512], in_=s0, func=AF.Exp,
                                 scale=SCALE)
            if j in ACT_H1:
                nc.scalar.activation(out=eb[:, 512:1024], in_=s1,
                                     func=AF.Exp, scale=SCALE)
            else:
                nc.vector.tensor_scalar(out=eT[:, 512:1024], in0=s1,
                                        scalar1=A_S, scalar2=B_S,
                                        op0=ALU.mult, op1=ALU.add)

        def emit_av(av, m, j, eT):
            eb = eT.bitcast(BF16)
            for qb in range(4):
                for h in range(2):
                    nc.tensor.matmul(
                        av[:, qb * 2 + h, 0:65],
                        lhsT=eb[:, h * 512 + qb * 128:
                                h * 512 + (qb + 1) * 128],
                        rhs=vT_sb[j][:, 2 * m + h, 0:65],
                        start=(j == 0), stop=(j == NKB - 1))

        def drain_a(av):
            """recip + normalize into avn (4 ACT + 4 DVE)."""
            rden = drp.tile([128, 8, 1], F32, tag="rden", name="rden",
                            bufs=2)
            nc.vector.reciprocal(rden, av[:, :, 64:65])
            avn = drp.tile([128, 4, 128], BF16, tag="avn", name="avn",
                           bufs=2)
            for qb in range(4):
                for h in range(2):
                    src = av[:, qb * 2 + h, 0:64]
                    dst = avn[:, qb, h * 64:(h + 1) * 64]
                    if (qb + h) % 2 == 0:
                        nc.scalar.activation(out=dst, in_=src, func=AF.Copy,
                                             scale=rden[:, qb * 2 + h, :])
                    else:
                        nc.vector.tensor_scalar(
                            out=dst, in0=src,
                            scalar1=rden[:, qb * 2 + h, :],
                            scalar2=None, op0=ALU.mult)
            return avn

        def drain_b(qc, m, avn):
            """transpose + opair copy (+v bias)."""
            tps_flat = new_ops().bitcast(BF16)[:, 0:512]
            tps = tps_flat.rearrange("p (qb q) -> p qb q", qb=4)
            for qb in range(4):
                nc.tensor.transpose(tps[:, qb, :], avn[:, qb, :], ident)
            op = drp.tile([128, 512], BF16, tag=f"op{m}", name=f"op{m}",
                          bufs=2)
            nc.vector.tensor_scalar(out=op, in0=tps_flat,
                                    scalar1=qkvb[:, 4 + m:5 + m],
                                    scalar2=None, op0=ALU.add)
            opair[(qc, m)] = op

        def emit_outproj(qc, m2):
            ps = new_ops()
            for kc in range(MT):
                nc.tensor.matmul(
                    ps, lhsT=woutT[kc][:, m2 * 128:(m2 + 1) * 128],
                    rhs=opair[(qc, kc)],
                    start=(kc == 0), stop=(kc == MT - 1))
            yt = drp.tile([128, 512], F32, tag="yt", name="yt", bufs=4)
            if m2 % 2 == 0:
                nc.scalar.activation(out=yt, in_=ps, func=AF.Copy)
            else:
                nc.vector.tensor_copy(yt, ps)
            nc.sync.dma_start(
                out=y_out[m2 * 128:(m2 + 1) * 128,
                          qc * 512:(qc + 1) * 512],
                in_=yt)

        prev = None          # (qc, m, av) awaiting drain
        pending_op = None    # qc awaiting outproj emission
        avn_pend = None
        for qc, m in pairs:
            av = new_av()
            eTs = {}
            for j in range(NKB):
                sS = emit_scores(qc, m, j)
                eT = expp.tile([128, 1024], I16, tag="eT", name="eT",
                               bufs=4)
                emit_exp(sS, eT, j)
                eTs[j] = eT
                if j == 0 and prev is not None:
                    avn_pend = drain_a(prev[2])
                if j == 1 and prev is not None:
                    drain_b(prev[0], prev[1], avn_pend)
                    avn_pend = None
                    if prev[1] == 1:
                        pending_op = prev[0]
                if j >= 2:
                    emit_av(av, m, j - 2, eTs.pop(j - 2))
                if pending_op is not None and 2 <= j <= 5:
                    emit_outproj(pending_op, j - 2)
                    if j == 5:
                        pending_op = None
            for j in (NKB - 2, NKB - 1):
                emit_av(av, m, j, eTs.pop(j))
            prev = (qc, m, av)
        # tail: drain last pair + final outproj
        drain_b(prev[0], prev[1], drain_a(prev[2]))
        for m2 in range(NT):
            emit_outproj(QC - 1, m2)

    nc.compile()
    return nc


_NC = None


def _get_nc():
    global _NC
    if _NC is None:
        _NC = _build()
    return _NC


def _gblk():
    g = np.zeros((128, 8), dtype=np.float32)
    for p in range(128):
        g[p, p // CPG] = 1.0
    return g


def kernel(x, gn_gamma, gn_beta, w_qkv, w_out, b_out, trace=False):
    x = np.asarray(x, dtype=np.float32)
    w_qkv = np.asarray(w_qkv, np.float32)
    w_out = np.asarray(w_out, np.float32)
    gbo = np.ascontiguousarray(np.stack(
        [np.asarray(gn_gamma, np.float32).reshape(C),
         np.asarray(gn_beta, np.float32).reshape(C)], axis=1))
    gblk = _gblk()
    gbt = np.ascontiguousarray(gblk.T)
    ident = np.eye(128, dtype=np.float32).astype(ml_dtypes.bfloat16)

    nc = _get_nc()
    in_maps = []
    for core in range(8):
        b, hg = core // 2, core % 2
        # wqkvT cols: [q m0, q m1, k m0, k m1, v m0, v m1] for local heads
        rows = np.concatenate([
            w_qkv[hg * 256:(hg + 1) * 256, :],
            w_qkv[C + hg * 256:C + (hg + 1) * 256, :],
            w_qkv[2 * C + hg * 256:2 * C + (hg + 1) * 256, :]], axis=0)
        wqkvT = np.ascontiguousarray(rows.T).astype(ml_dtypes.bfloat16)
        woutT = np.ascontiguousarray(
            w_out[:, hg * 256:(hg + 1) * 256].T).astype(ml_dtypes.bfloat16)
        in_maps.append({
            "x": np.ascontiguousarray(x[b]).astype(ml_dtypes.bfloat16),
            "wqkvT": wqkvT,
            "woutT": woutT,
            "gbo": gbo,
            "gblk": gblk,
            "gbt": gbt,
            "ident": ident,
        })
    res = run_bass_kernel_spmd(nc, in_maps, core_ids=list(range(8)),
                               trace=trace)
    y = np.empty((B, C, N), dtype=np.float32)
    bo = np.asarray(b_out, np.float32).reshape(C, 1)
    for b in range(B):
        y[b] = (res.results[2 * b]["y"] + res.results[2 * b + 1]["y"]
                + x[b] + bo)
    if trace:
        kernel.last_results = res
    return y


# revision 6
# speedup vs baseline: 1.2639x; 1.0024x over previous
"""EnhancedTemporalAttention Trainium2 kernel (v2).

Full module: GroupNorm(32) -> QKV 1x1conv -> 8-head attention (softmax) ->
out 1x1conv + bias -> +residual, on x [4, 512, 2048] fp32.

Sharding: 8 cores = (batch b = core//2) x (head-half hg = core%2).  Each
core computes GroupNorm stats + its 4 heads' Q/K/V over the full sequence,
attention for all 2048 queries, and a partial out-projection (contraction
over its 256 channels).  Host sums the two partials per batch and adds
residual + b_out exactly in fp32.

GroupNorm is folded into the QKV weights: w' = w * scale_c (per input
channel, scaled in place on Pool), plus a K=1 matvec for the bias term
which rides the projection PSUM->SBUF copies as a per-partition bias.

Attention uses transposed scores (keys on partitions, [k,q] layout); exp
splits between ACT (exact, 19/32 per pair) and DVE (Schraudolph int16
bit-trick into bf16 bits, 13/32).  AV runs with eT stationary / vT moving
so each matmul is only 65 output rows; softmax denominators ride a ones
column on vT; normalization is a per-partition multiply in [q,d] layout,
then a PE transpose (bf16) back to [c,q] for the out-projection, whose
PSUM result DMAs straight to DRAM.
"""
import sys

sys.path.insert(0, "/opt/trn_rl_repo")

import numpy as np
import ml_dtypes

import concourse.bacc as bacc
import concourse.bass as bass
import concourse.tile as tile
from concourse import mybir
from concourse.bass_utils import run_bass_kernel_spmd

F32 = mybir.dt.float32
F32R = mybir.dt.float32r
BF16 = mybir.dt.bfloat16
I16 = mybir.dt.int16

B = 4
C = 512
N = 2048
H = 8
HL = 4             # local heads per core
D = 64
G = 32             # groupnorm groups
CPG = C // G       # 16 channels per group
EPS = 1e-4
SCALE = D ** -0.5
NT = C // 128      # 4 input-channel tiles
MT = 2             # local qkv channel tiles (256 local channels)
NKB = N // 128     # 16 key blocks
QC = 4             # query chunks of 512
AF = mybir.ActivationFunctionType
ALU = mybir.AluOpType

# Schraudolph exp into bf16 bits: i16 = s*A_S + B_S, bitcast -> bf16
A_S = 184.6650085 * SCALE
B_S = 16249.1
# j's whose (whole-j, both-head) exp runs on ACT exactly; the rest run
# the Schraudolph bit-trick on DVE.
ACT_J = (0, 2, 4, 6, 8, 10, 12, 14, 15)


def _build(taps=False):
    nc = bacc.Bacc("TRN2", target_bir_lowering=False, debug=False)
    x_in = nc.dram_tensor("x", [C, N], BF16, kind="ExternalInput").ap()
    wqkvT_in = nc.dram_tensor("wqkvT", [C, 6 * 128], BF16,
                              kind="ExternalInput").ap()
    woutT_in = nc.dram_tensor("woutT", [MT * 128, C], BF16,
                              kind="ExternalInput").ap()
    gbo_in = nc.dram_tensor("gbo", [C, 2], F32, kind="ExternalInput").ap()
    gblk_in = nc.dram_tensor("gblk", [128, 8], F32, kind="ExternalInput").ap()
    gbt_in = nc.dram_tensor("gbt", [8, 128], F32, kind="ExternalInput").ap()
    id_in = nc.dram_tensor("ident", [128, 128], BF16,
                           kind="ExternalInput").ap()
    y_out = nc.dram_tensor("y", [C, N], F32, kind="ExternalOutput").ap()

    from contextlib import ExitStack
    with tile.TileContext(nc) as tc, ExitStack() as ctx:
        persist = ctx.enter_context(tc.tile_pool(name="persist", bufs=1))
        gn = ctx.enter_context(tc.tile_pool(name="gn", bufs=1))
        pspool = ctx.enter_context(tc.tile_pool(name="ps", bufs=1,
                                                space="PSUM"))
        expp = ctx.enter_context(tc.tile_pool(name="expp", bufs=1))
        drp = ctx.enter_context(tc.tile_pool(name="drp", bufs=1))

        # ---- persistent tiles ----
        X = [persist.tile([128, N], BF16, tag=f"X{t}", name=f"X{t}")
             for t in range(NT)]
        wT = [persist.tile([128, 6 * 128], BF16, tag=f"wT{kc}",
                           name=f"wT{kc}") for kc in range(NT)]
        woutT = [persist.tile([128, C], BF16, tag=f"woT{m}", name=f"woT{m}")
                 for m in range(MT)]
        q_sb = [persist.tile([128, N], BF16, tag=f"q{m}", name=f"q{m}")
                for m in range(MT)]
        k_sb = [persist.tile([128, N], BF16, tag=f"k{m}", name=f"k{m}")
                for m in range(MT)]
        vT_sb = [persist.tile([128, HL, 66], BF16, tag=f"vT{nb}",
                              name=f"vT{nb}") for nb in range(NKB)]
        ident = persist.tile([128, 128], BF16, tag="ident", name="ident")
        nc.sync.dma_start(out=ident, in_=id_in)

        # PSUM rings: S 3x4KB + av 4KB = 16KB exactly; every other
        # psum user (GN, projections, transpose, out-proj) shares the
        # S ring, using a [:, 0:512] half-slot view.
        def new_S():
            return pspool.tile([128, 1024], F32, tag="S", name="S", bufs=3)

        def new_ops():
            return new_S()[:, 0:512]

        def new_av():
            return pspool.tile([128, 8, 128], F32, tag="av", name="av",
                               bufs=1)

        # ---- input loads: x in 512-col chunks (bn_stats pipelines) ----
        for t in range(NT):
            for sg in range(4):
                nc.sync.dma_start(
                    out=X[t][:, sg * 512:(sg + 1) * 512],
                    in_=x_in[t * 128:(t + 1) * 128, sg * 512:(sg + 1) * 512])
        for kc in range(NT):
            nc.sync.dma_start(out=wT[kc],
                              in_=wqkvT_in[kc * 128:(kc + 1) * 128, :])
        for m in range(MT):
            nc.sync.dma_start(out=woutT[m],
                              in_=woutT_in[m * 128:(m + 1) * 128, :])
        gblk = gn.tile([128, 8], F32R, tag="gblk")
        nc.sync.dma_start(out=gblk, in_=gblk_in.bitcast(F32R))
        gbt = gn.tile([8, 128], F32R, tag="gbt")
        nc.sync.dma_start(out=gbt, in_=gbt_in.bitcast(F32R))
        gbo = []
        for t in range(NT):
            gt = gn.tile([128, 2], F32, tag=f"gbo{t}", name=f"gbo{t}")
            nc.sync.dma_start(out=gt, in_=gbo_in[t * 128:(t + 1) * 128, :])
            gbo.append(gt)

        # ---- GroupNorm stats ----
        eps_t = gn.tile([G, 1], F32, tag="eps_t")
        nc.vector.memset(eps_t, EPS)
        sqw = gn.tile([G, 1], F32, tag="sqw")
        nc.scalar.activation(out=sqw, in_=eps_t, func=AF.Sqrt)
        mvv = []
        for t in range(NT):
            stats = gn.tile([128, 4, 6], F32, tag=f"st{t}", name=f"st{t}")
            for sg in range(4):
                nc.vector.bn_stats(out=stats[:, sg, :],
                                   in_=X[t][:, sg * 512:(sg + 1) * 512])
            mv = gn.tile([128, 2], F32, tag=f"mv{t}", name=f"mv{t}")
            nc.vector.bn_aggr(out=mv, in_=stats)
            mt = gn.tile([128, 2], F32R, tag=f"mvv{t}", name=f"mvv{t}")
            nc.vector.tensor_copy(mt[:, 0:1], mv[:, 0:1])
            sqm = gn.tile([128, 1], F32, tag=f"sqm{t}", name=f"sqm{t}")
            nc.vector.tensor_mul(sqm, mv[:, 0:1], mv[:, 0:1])
            nc.vector.tensor_tensor(out=mt[:, 1:2], in0=mv[:, 1:2],
                                    in1=sqm, op=ALU.add)
            mvv.append(mt)
        g8ps = new_ops()
        for t in range(NT):
            nc.tensor.matmul(g8ps[0:8, t * 2:(t + 1) * 2],
                             lhsT=gblk, rhs=mvv[t],
                             start=(t == 0), stop=(t == NT - 1),
                             skip_group_check=True)
        g8 = gn.tile([8, NT, 2], F32, tag="g8")
        nc.vector.tensor_copy(g8.rearrange("p t s -> p (t s)"),
                              g8ps[0:8, 0:8])
        mean8 = gn.tile([8, NT], F32, tag="mean8")
        nc.vector.tensor_scalar_mul(mean8, g8[:, :, 0], 1.0 / CPG)
        ex28 = gn.tile([8, NT], F32, tag="ex28")
        nc.vector.tensor_scalar_mul(ex28, g8[:, :, 1], 1.0 / CPG)
        msq8 = gn.tile([8, NT], F32, tag="msq8")
        nc.vector.tensor_mul(msq8, mean8, mean8)
        var8 = gn.tile([8, NT], F32, tag="var8")
        nc.vector.tensor_tensor(out=var8, in0=ex28, in1=msq8,
                                op=ALU.subtract)
        std8 = gn.tile([8, NT], F32, tag="std8")
        nc.scalar.activation(out=std8, in_=var8, func=AF.Sqrt,
                             bias=eps_t[0:8, :])
        rstd8 = gn.tile([8, NT], F32, tag="rstd8")
        nc.vector.reciprocal(rstd8, std8)
        # preload the Exp table; chained after the real Sqrt via std8
        warm = gn.tile([8, NT], F32, tag="warm")
        nc.scalar.activation(out=warm, in_=std8, func=AF.Exp)
        mr8 = gn.tile([8, NT, 2], F32R, tag="mr8")
        nc.vector.tensor_copy(mr8[:, :, 0:1],
                              mean8.rearrange("p (t o) -> p t o", o=1))
        nc.vector.tensor_copy(mr8[:, :, 1:2],
                              rstd8.rearrange("p (t o) -> p t o", o=1))
        msps = new_ops()
        for t in range(NT):
            nc.tensor.matmul(msps[:, t * 2:(t + 1) * 2],
                             lhsT=gbt, rhs=mr8[:, t, :],
                             start=(t == 0), stop=(t == NT - 1),
                             skip_group_check=True)
        mscall = msps[:, 0:2 * NT].rearrange("p (t s) -> p t s", s=2)

        # per-channel scale_c = rstd*gamma, bias_c = beta - mean*scale_c
        qkvb_ps = None
        scale_c = []
        for t in range(NT):
            sc = gn.tile([128, 1], F32, tag=f"sc{t}", name=f"sc{t}")
            nc.vector.tensor_mul(sc, mscall[:, t, 1:2], gbo[t][:, 0:1])
            scale_c.append(sc)
            tmp = gn.tile([128, 1], F32, tag=f"tmp{t}", name=f"tmp{t}")
            nc.vector.tensor_mul(tmp, mscall[:, t, 0:1], sc)
            bias_c = gn.tile([128, 1], F32, tag=f"bc{t}", name=f"bc{t}")
            nc.vector.tensor_tensor(out=bias_c, in0=gbo[t][:, 1:2],
                                    in1=tmp, op=ALU.subtract)
            bb = gn.tile([128, 1], BF16, tag=f"bb{t}", name=f"bb{t}")
            nc.vector.tensor_copy(bb, bias_c)
            # qkv bias matvec against RAW weights (before scaling)
            if qkvb_ps is None:
                qkvb_ps = new_ops()
            for oc in range(6):
                nc.tensor.matmul(qkvb_ps[:, 6 * t + oc:6 * t + oc + 1],
                                 lhsT=wT[t][:, oc * 128:(oc + 1) * 128],
                                 rhs=bb, start=True, stop=True,
                                 skip_group_check=True)
        # reduce the 4 kc partials: qkvb[:, oc] = sum_t qkvb_ps[:, 6t+oc]
        qkvb_all = gn.tile([128, NT, 6], F32, tag="qkvb_all")
        nc.vector.tensor_copy(qkvb_all.rearrange("p t s -> p (t s)"),
                              qkvb_ps[:, 0:24])
        qkvb01 = gn.tile([128, 6], F32, tag="qkvb01")
        nc.vector.tensor_tensor(out=qkvb01, in0=qkvb_all[:, 0, :],
                                in1=qkvb_all[:, 1, :], op=ALU.add)
        qkvb23 = gn.tile([128, 6], F32, tag="qkvb23")
        nc.vector.tensor_tensor(out=qkvb23, in0=qkvb_all[:, 2, :],
                                in1=qkvb_all[:, 3, :], op=ALU.add)
        qkvb = gn.tile([128, 6], F32, tag="qkvb")
        nc.vector.tensor_tensor(out=qkvb, in0=qkvb01, in1=qkvb23,
                                op=ALU.add)
        # scale weights in place (per input-channel partition) on Pool
        for t in range(NT):
            nc.gpsimd.tensor_scalar(out=wT[t], in0=wT[t],
                                    scalar1=scale_c[t], scalar2=None,
                                    op0=ALU.mult)

        # ---- projections ----
        def kq_proj(which, m, ncx, eng):
            ps = new_ops()
            col0 = (m if which == "q" else 2 + m) * 128
            for kc in range(NT):
                nc.tensor.matmul(
                    ps, lhsT=wT[kc][:, col0:col0 + 128],
                    rhs=X[kc][:, ncx * 512:(ncx + 1) * 512],
                    start=(kc == 0), stop=(kc == NT - 1))
            dst = (q_sb if which == "q" else k_sb)[m][
                :, ncx * 512:(ncx + 1) * 512]
            boff = (0 if which == "q" else 2) + m
            if eng == "act":
                nc.scalar.activation(out=dst, in_=ps, func=AF.Identity,
                                     bias=qkvb[:, boff:boff + 1])
            else:
                nc.vector.tensor_scalar(out=dst, in0=ps,
                                        scalar1=qkvb[:, boff:boff + 1],
                                        scalar2=None, op0=ALU.add)

        def v_proj(nb, eng):
            ps = new_ops()
            for kc in range(NT):
                nc.tensor.matmul(
                    ps[:, 0:256], lhsT=X[kc][:, nb * 128:(nb + 1) * 128],
                    rhs=wT[kc][:, 4 * 128:6 * 128],
                    start=(kc == 0), stop=(kc == NT - 1))
            src = ps[:, 0:256].rearrange("p (h d) -> p h d", h=HL)
            dst = vT_sb[nb][:, :, 0:64]
            # v bias is folded at the opair copy; here plain convert
            if eng == "act":
                nc.scalar.activation(out=dst, in_=src, func=AF.Copy)
            else:
                nc.vector.tensor_copy(dst, src)
            nc.gpsimd.memset(vT_sb[nb][:, :, 64:65], 1.0)

        engs = ["act", "dve"]
        for m in range(MT):
            for ncx in range(4):
                kq_proj("k", m, ncx, engs[(m * 4 + ncx) % 2])
        for m in range(MT):
            for ncx in range(4):
                kq_proj("q", m, ncx, engs[(m * 4 + ncx + 1) % 2])
        for nb in range(NKB):
            v_proj(nb, engs[nb % 2])

        # ---- attention ----
        pairs = [(qc, m) for qc in range(QC) for m in range(MT)]
        opair = {}

        def emit_scores(qc, m, j):
            s = new_S()
            nc.tensor.matmul(
                s[:, 0:512], lhsT=k_sb[m][0:64, j * 128:(j + 1) * 128],
                rhs=q_sb[m][0:64, qc * 512:(qc + 1) * 512],
                start=True, stop=True, tile_position=(0, 0),
                skip_group_check=True)
            nc.tensor.matmul(
                s[:, 512:1024], lhsT=k_sb[m][64:128, j * 128:(j + 1) * 128],
                rhs=q_sb[m][64:128, qc * 512:(qc + 1) * 512],
                start=True, stop=True, tile_position=(64, 0),
                skip_group_check=True)
            return s

        def emit_exp(s, eT, j):
            if j in ACT_J:
                nc.scalar.activation(out=eT.bitcast(BF16), in_=s,
                                     func=AF.Exp, scale=SCALE)
            else:
                nc.vector.tensor_scalar(out=eT, in0=s,
                                        scalar1=A_S, scalar2=B_S,
                                        op0=ALU.mult, op1=ALU.add)

        def emit_av(av, m, j, eT):
            eb = eT.bitcast(BF16)
            for qb in range(4):
                for h in range(2):
                    nc.tensor.matmul(
                        av[:, qb * 2 + h, 0:65],
                        lhsT=eb[:, h * 512 + qb * 128:
                                h * 512 + (qb + 1) * 128],
                        rhs=vT_sb[j][:, 2 * m + h, 0:65],
                        start=(j == 0), stop=(j == NKB - 1))

        def drain_a(av):
            """recip + normalize into avn (4 ACT + 4 DVE)."""
            rden = drp.tile([128, 8, 1], F32, tag="rden", name="rden",
                            bufs=2)
            nc.vector.reciprocal(rden, av[:, :, 64:65])
            avn = drp.tile([128, 4, 128], BF16, tag="avn", name="avn",
                           bufs=2)
            for qb in range(4):
                for h in range(2):
                    src = av[:, qb * 2 + h, 0:64]
                    dst = avn[:, qb, h * 64:(h + 1) * 64]
                    if (qb + h) % 2 == 0:
                        nc.scalar.activation(out=dst, in_=src, func=AF.Copy,
                                             scale=rden[:, qb * 2 + h, :])
                    else:
                        nc.vector.tensor_scalar(
                            out=dst, in0=src,
                            scalar1=rden[:, qb * 2 + h, :],
                            scalar2=None, op0=ALU.mult)
            return avn

        def drain_b(qc, m, avn):
            """transpose + opair copy (+v bias)."""
            tps_flat = new_ops().bitcast(BF16)[:, 0:512]
            tps = tps_flat.rearrange("p (qb q) -> p qb q", qb=4)
            for qb in range(4):
                nc.tensor.transpose(tps[:, qb, :], avn[:, qb, :], ident)
            op = drp.tile([128, 512], BF16, tag=f"op{m}", name=f"op{m}",
                          bufs=2)
            nc.vector.tensor_scalar(out=op, in0=tps_flat,
                                    scalar1=qkvb[:, 4 + m:5 + m],
                                    scalar2=None, op0=ALU.add)
            opair[(qc, m)] = op

        def emit_outproj(qc, m2):
            ps = new_ops()
            for kc in range(MT):
                nc.tensor.matmul(
                    ps, lhsT=woutT[kc][:, m2 * 128:(m2 + 1) * 128],
                    rhs=opair[(qc, kc)],
                    start=(kc == 0), stop=(kc == MT - 1))
            yt = drp.tile([128, 512], F32, tag="yt", name="yt", bufs=4)
            if m2 % 2 == 0:
                nc.scalar.activation(out=yt, in_=ps, func=AF.Copy)
            else:
                nc.vector.tensor_copy(yt, ps)
            nc.sync.dma_start(
                out=y_out[m2 * 128:(m2 + 1) * 128,
                          qc * 512:(qc + 1) * 512],
                in_=yt)

        prev = None          # (qc, m, av) awaiting drain
        pending_op = None    # qc awaiting outproj emission
        avn_pend = None
        for qc, m in pairs:
            av = new_av()
            eTs = {}
            for j in range(NKB):
                s = emit_scores(qc, m, j)
                eT = expp.tile([128, 1024], I16, tag="eT", name="eT",
                               bufs=4)
                emit_exp(s, eT, j)
                eTs[j] = eT
                if j == 0 and prev is not None:
                    avn_pend = drain_a(prev[2])
                if j == 1 and prev is not None:
                    drain_b(prev[0], prev[1], avn_pend)
                    avn_pend = None
                    if prev[1] == 1:
                        pending_op = prev[0]
                if j >= 2:
                    emit_av(av, m, j - 2, eTs.pop(j - 2))
                if pending_op is not None and 2 <= j <= 5:
                    emit_outproj(pending_op, j - 2)
                    if j == 5:
                        pending_op = None
            for j in (NKB - 2, NKB - 1):
                emit_av(av, m, j, eTs.pop(j))
            prev = (qc, m, av)
        # tail: drain last pair + final outproj
        drain_b(prev[0], prev[1], drain_a(prev[2]))
        for m2 in range(NT):
            emit_outproj(QC - 1, m2)

    nc.compile()
    return nc


_NC = None


def _get_nc():
    global _NC
    if _NC is None:
        _NC = _build()
    return _NC


def _gblk():
    g = np.zeros((128, 8), dtype=np.float32)
    for p in range(128):
        g[p, p // CPG] = 1.0
    return g


def kernel(x, gn_gamma, gn_beta, w_qkv, w_out, b_out, trace=False):
    x = np.asarray(x, dtype=np.float32)
    w_qkv = np.asarray(w_qkv, np.float32)
    w_out = np.asarray(w_out, np.float32)
    gbo = np.ascontiguousarray(np.stack(
        [np.asarray(gn_gamma, np.float32).reshape(C),
         np.asarray(gn_beta, np.float32).reshape(C)], axis=1))
    gblk = _gblk()
    gbt = np.ascontiguousarray(gblk.T)
    ident = np.eye(128, dtype=np.float32).astype(ml_dtypes.bfloat16)

    nc = _get_nc()
    in_maps = []
    for core in range(8):
        b, hg = core // 2, core % 2
        # wqkvT cols: [q m0, q m1, k m0, k m1, v m0, v m1] for local heads
        rows = np.concatenate([
            w_qkv[hg * 256:(hg + 1) * 256, :],
            w_qkv[C + hg * 256:C + (hg + 1) * 256, :],
            w_qkv[2 * C + hg * 256:2 * C + (hg + 1) * 256, :]], axis=0)
        wqkvT = np.ascontiguousarray(rows.T).astype(ml_dtypes.bfloat16)
        woutT = np.ascontiguousarray(
            w_out[:, hg * 256:(hg + 1) * 256].T).astype(ml_dtypes.bfloat16)
        in_maps.append({
            "x": np.ascontiguousarray(x[b]).astype(ml_dtypes.bfloat16),
            "wqkvT": wqkvT,
            "woutT": woutT,
            "gbo": gbo,
            "gblk": gblk,
            "gbt": gbt,
            "ident": ident,
        })
    res = run_bass_kernel_spmd(nc, in_maps, core_ids=list(range(8)),
                               trace=trace)
    y = np.empty((B, C, N), dtype=np.float32)
    bo = np.asarray(b_out, np.float32).reshape(C, 1)
    for b in range(B):
        y[b] = (res.results[2 * b]["y"] + res.results[2 * b + 1]["y"]
                + x[b] + bo)
    if trace:
        kernel.last_results = res
    return y


# revision 11
# speedup vs baseline: 1.2925x; 1.0226x over previous
"""EnhancedTemporalAttention Trainium2 kernel (v2).

Full module: GroupNorm(32) -> QKV 1x1conv -> 8-head attention (softmax) ->
out 1x1conv + bias -> +residual, on x [4, 512, 2048] fp32.

Sharding: 8 cores = (batch b = core//2) x (head-half hg = core%2).  Each
core computes GroupNorm stats + its 4 heads' Q/K/V over the full sequence,
attention for all 2048 queries, and a partial out-projection (contraction
over its 256 channels).  Host sums the two partials per batch and adds
residual + b_out exactly in fp32.

GroupNorm is folded into the QKV weights: w' = w * scale_c (per input
channel, scaled in place on Pool), plus a K=1 matvec for the bias term
which rides the projection PSUM->SBUF copies as a per-partition bias.

Attention uses transposed scores (keys on partitions, [k,q] layout); exp
splits between ACT (exact, 19/32 per pair) and DVE (Schraudolph int16
bit-trick into bf16 bits, 13/32).  AV runs with eT stationary / vT moving
so each matmul is only 65 output rows; softmax denominators ride a ones
column on vT; normalization is a per-partition multiply in [q,d] layout,
then a PE transpose (bf16) back to [c,q] for the out-projection, whose
PSUM result DMAs straight to DRAM.
"""
import sys

sys.path.insert(0, "/opt/trn_rl_repo")

import numpy as np
import ml_dtypes

import concourse.bacc as bacc
import concourse.bass as bass
import concourse.tile as tile
from concourse import mybir
from concourse.bass_utils import run_bass_kernel_spmd

F32 = mybir.dt.float32
F32R = mybir.dt.float32r
BF16 = mybir.dt.bfloat16
I16 = mybir.dt.int16

B = 4
C = 512
N = 2048
H = 8
HL = 4             # local heads per core
D = 64
G = 32             # groupnorm groups
CPG = C // G       # 16 channels per group
EPS = 1e-4
SCALE = D ** -0.5
NT = C // 128      # 4 input-channel tiles
MT = 2             # local qkv channel tiles (256 local channels)
NKB = N // 128     # 16 key blocks
QC = 4             # query chunks of 512
AF = mybir.ActivationFunctionType
ALU = mybir.AluOpType

# Schraudolph exp into bf16 bits: i16 = s*A_S + B_S, bitcast -> bf16
A_S = 184.6650085 * SCALE
B_S = 16249.1
# j's whose (whole-j, both-head) exp runs on ACT exactly; the rest run
# the Schraudolph bit-trick on DVE.
ACT_J = (0, 2, 4, 6, 8, 10, 12, 14, 15)


def _build(taps=False):
    nc = bacc.Bacc("TRN2", target_bir_lowering=False, debug=False)
    x_in = nc.dram_tensor("x", [C, N], BF16, kind="ExternalInput").ap()
    wqkvT_in = nc.dram_tensor("wqkvT", [C, 6 * 128], BF16,
                              kind="ExternalInput").ap()
    woutT_in = nc.dram_tensor("woutT", [MT * 128, C], BF16,
                              kind="ExternalInput").ap()
    gbo_in = nc.dram_tensor("gbo", [128, 8], F32, kind="ExternalInput").ap()
    gblk_in = nc.dram_tensor("gblk", [128, 8], F32, kind="ExternalInput").ap()
    gbt_in = nc.dram_tensor("gbt", [8, 128], F32, kind="ExternalInput").ap()
    id_in = nc.dram_tensor("ident", [128, 128], BF16,
                           kind="ExternalInput").ap()
    y_out = nc.dram_tensor("y", [C, N], F32, kind="ExternalOutput").ap()

    from contextlib import ExitStack
    with tile.TileContext(nc) as tc, ExitStack() as ctx:
        persist = ctx.enter_context(tc.tile_pool(name="persist", bufs=1))
        gn = ctx.enter_context(tc.tile_pool(name="gn", bufs=1))
        pspool = ctx.enter_context(tc.tile_pool(name="ps", bufs=1,
                                                space="PSUM"))
        expp = ctx.enter_context(tc.tile_pool(name="expp", bufs=1))
        drp = ctx.enter_context(tc.tile_pool(name="drp", bufs=1))

        # ---- persistent tiles ----
        X = [persist.tile([128, N], BF16, tag=f"X{t}", name=f"X{t}")
             for t in range(NT)]
        wT = [persist.tile([128, 6 * 128], BF16, tag=f"wT{kc}",
                           name=f"wT{kc}") for kc in range(NT)]
        woutT = [persist.tile([128, C], BF16, tag=f"woT{m}", name=f"woT{m}")
                 for m in range(MT)]
        q_sb = [persist.tile([128, N], BF16, tag=f"q{m}", name=f"q{m}")
                for m in range(MT)]
        k_sb = [persist.tile([128, N], BF16, tag=f"k{m}", name=f"k{m}")
                for m in range(MT)]
        vT_sb = [persist.tile([128, HL, 66], BF16, tag=f"vT{nb}",
                              name=f"vT{nb}") for nb in range(NKB)]
        ident = persist.tile([128, 128], BF16, tag="ident", name="ident")

        # PSUM rings: S 3x4KB + av 4KB = 16KB exactly; every other
        # psum user (GN, projections, transpose, out-proj) shares the
        # S ring, using a [:, 0:512] half-slot view.
        def new_S():
            return pspool.tile([128, 1024], F32, tag="S", name="S", bufs=3)

        def new_ops():
            return new_S()[:, 0:512]

        def new_av():
            return pspool.tile([128, 8, 128], F32, tag="av", name="av",
                               bufs=1)

        # ---- input loads ----
        # x: 8 chunks of [128,1024] on the HWDGE path (critical for stats);
        # misc/gbt woven in after chunk 3.  Weights + ident ride the
        # software DGE (gpsimd) so they bypass the serialized HWDGE device.
        gbo4 = gn.tile([128, 8], F32, tag="gbo4")
        gblk = gn.tile([128, 8], F32R, tag="gblk")
        gbt = gn.tile([8, 128], F32R, tag="gbt")
        xchunks = [(t, half) for t in range(NT) for half in range(2)]
        for k, (t, half) in enumerate(xchunks):
            nc.sync.dma_start(
                out=X[t][:, half * 1024:(half + 1) * 1024],
                in_=x_in[t * 128:(t + 1) * 128,
                         half * 1024:(half + 1) * 1024])
            if k == 3:
                nc.sync.dma_start(out=gbo4, in_=gbo_in)
                nc.sync.dma_start(out=gblk, in_=gblk_in.bitcast(F32R))
                nc.sync.dma_start(out=gbt, in_=gbt_in.bitcast(F32R))
        for kc in range(NT):
            nc.scalar.dma_start(out=wT[kc],
                                in_=wqkvT_in[kc * 128:(kc + 1) * 128, :])
        for m in range(MT):
            nc.scalar.dma_start(out=woutT[m],
                                in_=woutT_in[m * 128:(m + 1) * 128, :])
        nc.scalar.dma_start(out=ident, in_=id_in)
        gbo = [gbo4[:, 2 * t:2 * t + 2] for t in range(NT)]

        # ---- GroupNorm stats ----
        eps_t = gn.tile([G, 1], F32, tag="eps_t")
        nc.vector.memset(eps_t, EPS)
        sqw = gn.tile([G, 1], F32, tag="sqw")
        nc.scalar.activation(out=sqw, in_=eps_t, func=AF.Sqrt)
        mvv = []
        for t in range(NT):
            stats = gn.tile([128, 4, 6], F32, tag=f"st{t}", name=f"st{t}")
            for sg in range(4):
                nc.vector.bn_stats(out=stats[:, sg, :],
                                   in_=X[t][:, sg * 512:(sg + 1) * 512])
            mv = gn.tile([128, 2], F32, tag=f"mv{t}", name=f"mv{t}")
            nc.vector.bn_aggr(out=mv, in_=stats)
            mt = gn.tile([128, 2], F32R, tag=f"mvv{t}", name=f"mvv{t}")
            nc.vector.tensor_copy(mt[:, 0:1], mv[:, 0:1])
            sqm = gn.tile([128, 1], F32, tag=f"sqm{t}", name=f"sqm{t}")
            nc.vector.tensor_mul(sqm, mv[:, 0:1], mv[:, 0:1])
            nc.vector.tensor_tensor(out=mt[:, 1:2], in0=mv[:, 1:2],
                                    in1=sqm, op=ALU.add)
            mvv.append(mt)
        g8ps = new_ops()
        for t in range(NT):
            nc.tensor.matmul(g8ps[0:8, t * 2:(t + 1) * 2],
                             lhsT=gblk, rhs=mvv[t],
                             start=(t == 0), stop=(t == NT - 1),
                             skip_group_check=True)
        g8 = gn.tile([8, NT, 2], F32, tag="g8")
        nc.vector.tensor_copy(g8.rearrange("p t s -> p (t s)"),
                              g8ps[0:8, 0:8])
        mean8 = gn.tile([8, NT], F32, tag="mean8")
        nc.vector.tensor_scalar_mul(mean8, g8[:, :, 0], 1.0 / CPG)
        ex28 = gn.tile([8, NT], F32, tag="ex28")
        nc.vector.tensor_scalar_mul(ex28, g8[:, :, 1], 1.0 / CPG)
        msq8 = gn.tile([8, NT], F32, tag="msq8")
        nc.vector.tensor_mul(msq8, mean8, mean8)
        var8 = gn.tile([8, NT], F32, tag="var8")
        nc.vector.tensor_tensor(out=var8, in0=ex28, in1=msq8,
                                op=ALU.subtract)
        std8 = gn.tile([8, NT], F32, tag="std8")
        nc.scalar.activation(out=std8, in_=var8, func=AF.Sqrt,
                             bias=eps_t[0:8, :])
        rstd8 = gn.tile([8, NT], F32, tag="rstd8")
        nc.vector.reciprocal(rstd8, std8)
        # preload the Exp table; chained after the real Sqrt via std8
        warm = gn.tile([8, NT], F32, tag="warm")
        nc.scalar.activation(out=warm, in_=std8, func=AF.Exp)
        mr8 = gn.tile([8, NT, 2], F32R, tag="mr8")
        nc.vector.tensor_copy(mr8[:, :, 0:1],
                              mean8.rearrange("p (t o) -> p t o", o=1))
        nc.vector.tensor_copy(mr8[:, :, 1:2],
                              rstd8.rearrange("p (t o) -> p t o", o=1))
        msps = new_ops()
        for t in range(NT):
            nc.tensor.matmul(msps[:, t * 2:(t + 1) * 2],
                             lhsT=gbt, rhs=mr8[:, t, :],
                             start=(t == 0), stop=(t == NT - 1),
                             skip_group_check=True)
        mscall = msps[:, 0:2 * NT].rearrange("p (t s) -> p t s", s=2)

        # per-channel scale_c = rstd*gamma, bias_c = beta - mean*scale_c
        qkvb_ps = None
        scale_c = []
        for t in range(NT):
            sc = gn.tile([128, 1], F32, tag=f"sc{t}", name=f"sc{t}")
            nc.vector.tensor_mul(sc, mscall[:, t, 1:2], gbo[t][:, 0:1])
            scale_c.append(sc)
            tmp = gn.tile([128, 1], F32, tag=f"tmp{t}", name=f"tmp{t}")
            nc.vector.tensor_mul(tmp, mscall[:, t, 0:1], sc)
            bias_c = gn.tile([128, 1], F32, tag=f"bc{t}", name=f"bc{t}")
            nc.vector.tensor_tensor(out=bias_c, in0=gbo[t][:, 1:2],
                                    in1=tmp, op=ALU.subtract)
            bb = gn.tile([128, 1], BF16, tag=f"bb{t}", name=f"bb{t}")
            nc.vector.tensor_copy(bb, bias_c)
            # qkv bias matvec against RAW weights (before scaling)
            if qkvb_ps is None:
                qkvb_ps = new_ops()
            for oc in range(6):
                nc.tensor.matmul(qkvb_ps[:, 6 * t + oc:6 * t + oc + 1],
                                 lhsT=wT[t][:, oc * 128:(oc + 1) * 128],
                                 rhs=bb, start=True, stop=True,
                                 skip_group_check=True)
        # reduce the 4 kc partials: qkvb[:, oc] = sum_t qkvb_ps[:, 6t+oc]
        qkvb_all = gn.tile([128, NT, 6], F32, tag="qkvb_all")
        nc.vector.tensor_copy(qkvb_all.rearrange("p t s -> p (t s)"),
                              qkvb_ps[:, 0:24])
        qkvb01 = gn.tile([128, 6], F32, tag="qkvb01")
        nc.vector.tensor_tensor(out=qkvb01, in0=qkvb_all[:, 0, :],
                                in1=qkvb_all[:, 1, :], op=ALU.add)
        qkvb23 = gn.tile([128, 6], F32, tag="qkvb23")
        nc.vector.tensor_tensor(out=qkvb23, in0=qkvb_all[:, 2, :],
                                in1=qkvb_all[:, 3, :], op=ALU.add)
        qkvb = gn.tile([128, 6], F32, tag="qkvb")
        nc.vector.tensor_tensor(out=qkvb, in0=qkvb01, in1=qkvb23,
                                op=ALU.add)
        # scale weights in place (per input-channel partition) on Pool
        for t in range(NT):
            nc.gpsimd.tensor_scalar(out=wT[t], in0=wT[t],
                                    scalar1=scale_c[t], scalar2=None,
                                    op0=ALU.mult)

        # ---- projections ----
        def kq_proj(which, m, ncx, eng):
            ps = new_ops()
            col0 = (m if which == "q" else 2 + m) * 128
            for kc in range(NT):
                nc.tensor.matmul(
                    ps, lhsT=wT[kc][:, col0:col0 + 128],
                    rhs=X[kc][:, ncx * 512:(ncx + 1) * 512],
                    start=(kc == 0), stop=(kc == NT - 1))
            dst = (q_sb if which == "q" else k_sb)[m][
                :, ncx * 512:(ncx + 1) * 512]
            boff = (0 if which == "q" else 2) + m
            if eng == "act":
                nc.scalar.activation(out=dst, in_=ps, func=AF.Identity,
                                     bias=qkvb[:, boff:boff + 1])
            else:
                nc.vector.tensor_scalar(out=dst, in0=ps,
                                        scalar1=qkvb[:, boff:boff + 1],
                                        scalar2=None, op0=ALU.add)

        def v_proj(nb, eng):
            ps = new_ops()
            for kc in range(NT):
                nc.tensor.matmul(
                    ps[:, 0:256], lhsT=X[kc][:, nb * 128:(nb + 1) * 128],
                    rhs=wT[kc][:, 4 * 128:6 * 128],
                    start=(kc == 0), stop=(kc == NT - 1))
            src = ps[:, 0:256].rearrange("p (h d) -> p h d", h=HL)
            dst = vT_sb[nb][:, :, 0:64]
            # v bias is folded at the opair copy; here plain convert
            if eng == "act":
                nc.scalar.activation(out=dst, in_=src, func=AF.Copy)
            else:
                nc.vector.tensor_copy(dst, src)
            nc.gpsimd.memset(vT_sb[nb][:, :, 64:65], 1.0)

        engs = ["act", "dve"]
        for m in range(MT):
            for ncx in range(4):
                kq_proj("k", m, ncx, engs[(m * 4 + ncx) % 2])
        for m in range(MT):
            for ncx in range(4):
                kq_proj("q", m, ncx, engs[(m * 4 + ncx + 1) % 2])
        for nb in range(NKB):
            v_proj(nb, engs[nb % 2])

        # ---- attention ----
        pairs = [(qc, m) for qc in range(QC) for m in range(MT)]
        opair = {}

        def emit_scores(qc, m, j):
            s = new_S()
            nc.tensor.matmul(
                s[:, 0:512], lhsT=k_sb[m][0:64, j * 128:(j + 1) * 128],
                rhs=q_sb[m][0:64, qc * 512:(qc + 1) * 512],
                start=True, stop=True, tile_position=(0, 0),
                skip_group_check=True)
            nc.tensor.matmul(
                s[:, 512:1024], lhsT=k_sb[m][64:128, j * 128:(j + 1) * 128],
                rhs=q_sb[m][64:128, qc * 512:(qc + 1) * 512],
                start=True, stop=True, tile_position=(64, 0),
                skip_group_check=True)
            return s

        def emit_exp(s, eT, j):
            if j in ACT_J:
                nc.scalar.activation(out=eT.bitcast(BF16), in_=s,
                                     func=AF.Exp, scale=SCALE)
            else:
                nc.vector.tensor_scalar(out=eT, in0=s,
                                        scalar1=A_S, scalar2=B_S,
                                        op0=ALU.mult, op1=ALU.add)

        def emit_av(av, m, j, eT):
            eb = eT.bitcast(BF16)
            for qb in range(4):
                for h in range(2):
                    nc.tensor.matmul(
                        av[:, qb * 2 + h, 0:65],
                        lhsT=eb[:, h * 512 + qb * 128:
                                h * 512 + (qb + 1) * 128],
                        rhs=vT_sb[j][:, 2 * m + h, 0:65],
                        start=(j == 0), stop=(j == NKB - 1))

        def drain_a(av):
            """recip + normalize into avn (4 ACT + 4 DVE)."""
            rden = drp.tile([128, 8, 1], F32, tag="rden", name="rden",
                            bufs=2)
            nc.vector.reciprocal(rden, av[:, :, 64:65])
            avn = drp.tile([128, 4, 128], BF16, tag="avn", name="avn",
                           bufs=2)
            for qb in range(4):
                for h in range(2):
                    src = av[:, qb * 2 + h, 0:64]
                    dst = avn[:, qb, h * 64:(h + 1) * 64]
                    if (qb + h) % 2 == 0:
                        nc.scalar.activation(out=dst, in_=src, func=AF.Copy,
                                             scale=rden[:, qb * 2 + h, :])
                    else:
                        nc.vector.tensor_scalar(
                            out=dst, in0=src,
                            scalar1=rden[:, qb * 2 + h, :],
                            scalar2=None, op0=ALU.mult)
            return avn

        def drain_b(qc, m, avn):
            """transpose + opair copy (+v bias)."""
            tps_flat = new_ops().bitcast(BF16)[:, 0:512]
            tps = tps_flat.rearrange("p (qb q) -> p qb q", qb=4)
            for qb in range(4):
                nc.tensor.transpose(tps[:, qb, :], avn[:, qb, :], ident)
            op = drp.tile([128, 512], BF16, tag=f"op{m}", name=f"op{m}",
                          bufs=2)
            nc.vector.tensor_scalar(out=op, in0=tps_flat,
                                    scalar1=qkvb[:, 4 + m:5 + m],
                                    scalar2=None, op0=ALU.add)
            opair[(qc, m)] = op

        def emit_outproj(qc, m2):
            ps = new_ops()
            for kc in range(MT):
                nc.tensor.matmul(
                    ps, lhsT=woutT[kc][:, m2 * 128:(m2 + 1) * 128],
                    rhs=opair[(qc, kc)],
                    start=(kc == 0), stop=(kc == MT - 1))
            yt = drp.tile([128, 512], F32, tag="yt", name="yt", bufs=4)
            if m2 % 2 == 0:
                nc.scalar.activation(out=yt, in_=ps, func=AF.Copy)
            else:
                nc.vector.tensor_copy(yt, ps)
            nc.sync.dma_start(
                out=y_out[m2 * 128:(m2 + 1) * 128,
                          qc * 512:(qc + 1) * 512],
                in_=yt)

        prev = None          # (qc, m, av) awaiting drain
        pending_op = None    # qc awaiting outproj emission
        avn_pend = None
        for qc, m in pairs:
            av = new_av()
            eTs = {}
            opj = {4: 0, 7: 1, 10: 2, 13: 3}
            for j in range(NKB):
                s = emit_scores(qc, m, j)
                eT = expp.tile([128, 1024], I16, tag="eT", name="eT",
                               bufs=4)
                if j == 0 and prev is not None:
                    avn_pend = drain_a(prev[2])
                emit_exp(s, eT, j)
                eTs[j] = eT
                if j == 2 and prev is not None:
                    drain_b(prev[0], prev[1], avn_pend)
                    avn_pend = None
                    if prev[1] == 1:
                        pending_op = prev[0]
                if j >= 2:
                    emit_av(av, m, j - 2, eTs.pop(j - 2))
                if pending_op is not None and j in opj:
                    emit_outproj(pending_op, opj[j])
                    if j == 13:
                        pending_op = None
            for j in (NKB - 2, NKB - 1):
                emit_av(av, m, j, eTs.pop(j))
            prev = (qc, m, av)
        # tail: drain last pair + final outproj
        drain_b(prev[0], prev[1], drain_a(prev[2]))
        for m2 in range(NT):
            emit_outproj(QC - 1, m2)

    nc.compile()
    return nc


_NC = None


def _get_nc():
    global _NC
    if _NC is None:
        _NC = _build()
    return _NC


def _gblk():
    g = np.zeros((128, 8), dtype=np.float32)
    for p in range(128):
        g[p, p // CPG] = 1.0
    return g


def kernel(x, gn_gamma, gn_beta, w_qkv, w_out, b_out, trace=False):
    x = np.asarray(x, dtype=np.float32)
    w_qkv = np.asarray(w_qkv, np.float32)
    w_out = np.asarray(w_out, np.float32)
    gblk = _gblk()
    gbt = np.ascontiguousarray(gblk.T)
    gamma = np.asarray(gn_gamma, np.float32).reshape(C)
    beta = np.asarray(gn_beta, np.float32).reshape(C)
    gbo4 = np.zeros((128, 8), dtype=np.float32)
    for t in range(4):
        gbo4[:, 2 * t] = gamma[t * 128:(t + 1) * 128]
        gbo4[:, 2 * t + 1] = beta[t * 128:(t + 1) * 128]
    ident = np.eye(128, dtype=np.float32).astype(ml_dtypes.bfloat16)

    nc = _get_nc()
    in_maps = []
    for core in range(8):
        b, hg = core // 2, core % 2
        # wqkvT cols: [q m0, q m1, k m0, k m1, v m0, v m1] for local heads
        rows = np.concatenate([
            w_qkv[hg * 256:(hg + 1) * 256, :],
            w_qkv[C + hg * 256:C + (hg + 1) * 256, :],
            w_qkv[2 * C + hg * 256:2 * C + (hg + 1) * 256, :]], axis=0)
        wqkvT = np.ascontiguousarray(rows.T).astype(ml_dtypes.bfloat16)
        woutT = np.ascontiguousarray(
            w_out[:, hg * 256:(hg + 1) * 256].T).astype(ml_dtypes.bfloat16)
        in_maps.append({
            "x": np.ascontiguousarray(x[b]).astype(ml_dtypes.bfloat16),
            "wqkvT": wqkvT,
            "woutT": woutT,
            "gbo": gbo4,
            "gblk": gblk,
            "gbt": gbt,
            "ident": ident,
        })
    res = run_bass_kernel_spmd(nc, in_maps, core_ids=list(range(8)),
                               trace=trace)
    y = np.empty((B, C, N), dtype=np.float32)
    bo = np.asarray(b_out, np.float32).reshape(C, 1)
    for b in range(B):
        y[b] = (res.results[2 * b]["y"] + res.results[2 * b + 1]["y"]
                + x[b] + bo)
    if trace:
        kernel.last_results = res
    return y


# revision 12
# speedup vs baseline: 1.3053x; 1.0099x over previous
"""EnhancedTemporalAttention Trainium2 kernel (v2).

Full module: GroupNorm(32) -> QKV 1x1conv -> 8-head attention (softmax) ->
out 1x1conv + bias -> +residual, on x [4, 512, 2048] fp32.

Sharding: 8 cores = (batch b = core//2) x (head-half hg = core%2).  Each
core computes GroupNorm stats + its 4 heads' Q/K/V over the full sequence,
attention for all 2048 queries, and a partial out-projection (contraction
over its 256 channels).  Host sums the two partials per batch and adds
residual + b_out exactly in fp32.

GroupNorm is folded into the QKV weights: w' = w * scale_c (per input
channel, scaled in place on Pool), plus a K=1 matvec for the bias term
which rides the projection PSUM->SBUF copies as a per-partition bias.

Attention uses transposed scores (keys on partitions, [k,q] layout); exp
splits between ACT (exact, 19/32 per pair) and DVE (Schraudolph int16
bit-trick into bf16 bits, 13/32).  AV runs with eT stationary / vT moving
so each matmul is only 65 output rows; softmax denominators ride a ones
column on vT; normalization is a per-partition multiply in [q,d] layout,
then a PE transpose (bf16) back to [c,q] for the out-projection, whose
PSUM result DMAs straight to DRAM.
"""
import sys

sys.path.insert(0, "/opt/trn_rl_repo")

import numpy as np
import ml_dtypes

import concourse.bacc as bacc
import concourse.bass as bass
import concourse.tile as tile
from concourse import mybir
from concourse.bass_utils import run_bass_kernel_spmd

F32 = mybir.dt.float32
F32R = mybir.dt.float32r
BF16 = mybir.dt.bfloat16
I16 = mybir.dt.int16

B = 4
C = 512
N = 2048
H = 8
HL = 4             # local heads per core
D = 64
G = 32             # groupnorm groups
CPG = C // G       # 16 channels per group
EPS = 1e-4
SCALE = D ** -0.5
NT = C // 128      # 4 input-channel tiles
MT = 2             # local qkv channel tiles (256 local channels)
NKB = N // 128     # 16 key blocks
QC = 4             # query chunks of 512
AF = mybir.ActivationFunctionType
ALU = mybir.AluOpType

# Schraudolph exp into bf16 bits: i16 = s*A_S + B_S, bitcast -> bf16
A_S = 184.6650085 * SCALE
B_S = 16249.1
# j's whose (whole-j, both-head) exp runs on ACT exactly; the rest run
# the Schraudolph bit-trick on DVE.
ACT_J = (0, 2, 4, 6, 8, 10, 12, 14, 15)


def _build(taps=False):
    nc = bacc.Bacc("TRN2", target_bir_lowering=False, debug=False)
    x_in = nc.dram_tensor("x", [C, N], BF16, kind="ExternalInput").ap()
    wqkvT_in = nc.dram_tensor("wqkvT", [C, 6 * 128], BF16,
                              kind="ExternalInput").ap()
    woutT_in = nc.dram_tensor("woutT", [MT * 128, C], BF16,
                              kind="ExternalInput").ap()
    gbo_in = nc.dram_tensor("gbo", [128, 8], F32, kind="ExternalInput").ap()
    gblk_in = nc.dram_tensor("gblk", [128, 8], F32, kind="ExternalInput").ap()
    gbt_in = nc.dram_tensor("gbt", [8, 128], F32, kind="ExternalInput").ap()
    id_in = nc.dram_tensor("ident", [128, 128], BF16,
                           kind="ExternalInput").ap()
    y_out = nc.dram_tensor("y", [C, N], F32, kind="ExternalOutput").ap()

    from contextlib import ExitStack
    with tile.TileContext(nc) as tc, ExitStack() as ctx:
        persist = ctx.enter_context(tc.tile_pool(name="persist", bufs=1))
        gn = ctx.enter_context(tc.tile_pool(name="gn", bufs=1))
        pspool = ctx.enter_context(tc.tile_pool(name="ps", bufs=1,
                                                space="PSUM"))
        expp = ctx.enter_context(tc.tile_pool(name="expp", bufs=1))
        drp = ctx.enter_context(tc.tile_pool(name="drp", bufs=1))

        # ---- persistent tiles ----
        X = [persist.tile([128, N], BF16, tag=f"X{t}", name=f"X{t}")
             for t in range(NT)]
        wT = [persist.tile([128, 6 * 128], BF16, tag=f"wT{kc}",
                           name=f"wT{kc}") for kc in range(NT)]
        woutT = [persist.tile([128, C], BF16, tag=f"woT{m}", name=f"woT{m}")
                 for m in range(MT)]
        q_sb = [persist.tile([128, N], BF16, tag=f"q{m}", name=f"q{m}")
                for m in range(MT)]
        k_sb = [persist.tile([128, N], BF16, tag=f"k{m}", name=f"k{m}")
                for m in range(MT)]
        vT_sb = [persist.tile([128, HL, 66], BF16, tag=f"vT{nb}",
                              name=f"vT{nb}") for nb in range(NKB)]
        ident = persist.tile([128, 128], BF16, tag="ident", name="ident")

        # PSUM rings: S 3x4KB + av 4KB = 16KB exactly; every other
        # psum user (GN, projections, transpose, out-proj) shares the
        # S ring, using a [:, 0:512] half-slot view.
        def new_S():
            return pspool.tile([128, 1024], F32, tag="S", name="S", bufs=3)

        def new_ops():
            return new_S()[:, 0:512]

        def new_av():
            return pspool.tile([128, 8, 128], F32, tag="av", name="av",
                               bufs=1)

        # ---- input loads ----
        # x: 8 chunks of [128,1024] on the HWDGE path (critical for stats);
        # misc/gbt woven in after chunk 3.  Weights + ident ride the
        # software DGE (gpsimd) so they bypass the serialized HWDGE device.
        gbo4 = gn.tile([128, 8], F32, tag="gbo4")
        gblk = gn.tile([128, 8], F32R, tag="gblk")
        gbt = gn.tile([8, 128], F32R, tag="gbt")
        for t in range(NT):
            for half in range(2):
                nc.sync.dma_start(
                    out=X[t][:, half * 1024:(half + 1) * 1024],
                    in_=x_in[t * 128:(t + 1) * 128,
                             half * 1024:(half + 1) * 1024])
        nc.sync.dma_start(out=gbo4, in_=gbo_in)
        nc.sync.dma_start(out=gblk, in_=gblk_in.bitcast(F32R))
        nc.sync.dma_start(out=gbt, in_=gbt_in.bitcast(F32R))
        for kc in range(NT):
            nc.sync.dma_start(out=wT[kc],
                              in_=wqkvT_in[kc * 128:(kc + 1) * 128, :])
        for m in range(MT):
            nc.sync.dma_start(out=woutT[m],
                              in_=woutT_in[m * 128:(m + 1) * 128, :])
        nc.sync.dma_start(out=ident, in_=id_in)
        gbo = [gbo4[:, 2 * t:2 * t + 2] for t in range(NT)]

        # ---- GroupNorm stats ----
        eps_t = gn.tile([G, 1], F32, tag="eps_t")
        nc.vector.memset(eps_t, EPS)
        sqw = gn.tile([G, 1], F32, tag="sqw")
        nc.scalar.activation(out=sqw, in_=eps_t, func=AF.Sqrt)
        mvv = []
        for t in range(NT):
            stats = gn.tile([128, 4, 6], F32, tag=f"st{t}", name=f"st{t}")
            for sg in range(4):
                nc.vector.bn_stats(out=stats[:, sg, :],
                                   in_=X[t][:, sg * 512:(sg + 1) * 512])
            mv = gn.tile([128, 2], F32, tag=f"mv{t}", name=f"mv{t}")
            nc.vector.bn_aggr(out=mv, in_=stats)
            mt = gn.tile([128, 2], F32R, tag=f"mvv{t}", name=f"mvv{t}")
            nc.vector.tensor_copy(mt[:, 0:1], mv[:, 0:1])
            sqm = gn.tile([128, 1], F32, tag=f"sqm{t}", name=f"sqm{t}")
            nc.vector.tensor_mul(sqm, mv[:, 0:1], mv[:, 0:1])
            nc.vector.tensor_tensor(out=mt[:, 1:2], in0=mv[:, 1:2],
                                    in1=sqm, op=ALU.add)
            mvv.append(mt)
        g8ps = new_ops()
        for t in range(NT):
            nc.tensor.matmul(g8ps[0:8, t * 2:(t + 1) * 2],
                             lhsT=gblk, rhs=mvv[t],
                             start=(t == 0), stop=(t == NT - 1),
                             skip_group_check=True)
        g8 = gn.tile([8, NT, 2], F32, tag="g8")
        nc.vector.tensor_copy(g8.rearrange("p t s -> p (t s)"),
                              g8ps[0:8, 0:8])
        mean8 = gn.tile([8, NT], F32, tag="mean8")
        nc.vector.tensor_scalar_mul(mean8, g8[:, :, 0], 1.0 / CPG)
        ex28 = gn.tile([8, NT], F32, tag="ex28")
        nc.vector.tensor_scalar_mul(ex28, g8[:, :, 1], 1.0 / CPG)
        msq8 = gn.tile([8, NT], F32, tag="msq8")
        nc.vector.tensor_mul(msq8, mean8, mean8)
        var8 = gn.tile([8, NT], F32, tag="var8")
        nc.vector.tensor_tensor(out=var8, in0=ex28, in1=msq8,
                                op=ALU.subtract)
        std8 = gn.tile([8, NT], F32, tag="std8")
        nc.scalar.activation(out=std8, in_=var8, func=AF.Sqrt,
                             bias=eps_t[0:8, :])
        rstd8 = gn.tile([8, NT], F32, tag="rstd8")
        nc.vector.reciprocal(rstd8, std8)
        # preload the Exp table; chained after the real Sqrt via std8
        warm = gn.tile([8, NT], F32, tag="warm")
        nc.scalar.activation(out=warm, in_=std8, func=AF.Exp)
        mr8 = gn.tile([8, NT, 2], F32R, tag="mr8")
        nc.vector.tensor_copy(mr8[:, :, 0:1],
                              mean8.rearrange("p (t o) -> p t o", o=1))
        nc.vector.tensor_copy(mr8[:, :, 1:2],
                              rstd8.rearrange("p (t o) -> p t o", o=1))
        msps = new_ops()
        for t in range(NT):
            nc.tensor.matmul(msps[:, t * 2:(t + 1) * 2],
                             lhsT=gbt, rhs=mr8[:, t, :],
                             start=(t == 0), stop=(t == NT - 1),
                             skip_group_check=True)
        mscall = msps[:, 0:2 * NT].rearrange("p (t s) -> p t s", s=2)

        # per-channel scale_c = rstd*gamma, bias_c = beta - mean*scale_c
        qkvb_ps = None
        scale_c = []
        for t in range(NT):
            sc = gn.tile([128, 1], F32, tag=f"sc{t}", name=f"sc{t}")
            nc.vector.tensor_mul(sc, mscall[:, t, 1:2], gbo[t][:, 0:1])
            scale_c.append(sc)
            tmp = gn.tile([128, 1], F32, tag=f"tmp{t}", name=f"tmp{t}")
            nc.vector.tensor_mul(tmp, mscall[:, t, 0:1], sc)
            bias_c = gn.tile([128, 1], F32, tag=f"bc{t}", name=f"bc{t}")
            nc.vector.tensor_tensor(out=bias_c, in0=gbo[t][:, 1:2],
                                    in1=tmp, op=ALU.subtract)
            bb = gn.tile([128, 1], BF16, tag=f"bb{t}", name=f"bb{t}")
            nc.vector.tensor_copy(bb, bias_c)
            # qkv bias matvec against RAW weights (before scaling)
            if qkvb_ps is None:
                qkvb_ps = new_ops()
            for oc in range(6):
                nc.tensor.matmul(qkvb_ps[:, 6 * t + oc:6 * t + oc + 1],
                                 lhsT=wT[t][:, oc * 128:(oc + 1) * 128],
                                 rhs=bb, start=True, stop=True,
                                 skip_group_check=True)
        # reduce the 4 kc partials: qkvb[:, oc] = sum_t qkvb_ps[:, 6t+oc]
        qkvb_all = gn.tile([128, NT, 6], F32, tag="qkvb_all")
        nc.vector.tensor_copy(qkvb_all.rearrange("p t s -> p (t s)"),
                              qkvb_ps[:, 0:24])
        qkvb01 = gn.tile([128, 6], F32, tag="qkvb01")
        nc.vector.tensor_tensor(out=qkvb01, in0=qkvb_all[:, 0, :],
                                in1=qkvb_all[:, 1, :], op=ALU.add)
        qkvb23 = gn.tile([128, 6], F32, tag="qkvb23")
        nc.vector.tensor_tensor(out=qkvb23, in0=qkvb_all[:, 2, :],
                                in1=qkvb_all[:, 3, :], op=ALU.add)
        qkvb = gn.tile([128, 6], F32, tag="qkvb")
        nc.vector.tensor_tensor(out=qkvb, in0=qkvb01, in1=qkvb23,
                                op=ALU.add)
        # scale weights in place (per input-channel partition)
        for t in range(NT):
            if t < 2:
                nc.scalar.activation(out=wT[t], in_=wT[t], func=AF.Copy,
                                     scale=scale_c[t])
            else:
                nc.gpsimd.tensor_scalar(out=wT[t], in0=wT[t],
                                        scalar1=scale_c[t], scalar2=None,
                                        op0=ALU.mult)

        # ---- projections ----
        def kq_proj(which, m, ncx, eng):
            ps = new_ops()
            col0 = (m if which == "q" else 2 + m) * 128
            for kc in range(NT):
                nc.tensor.matmul(
                    ps, lhsT=wT[kc][:, col0:col0 + 128],
                    rhs=X[kc][:, ncx * 512:(ncx + 1) * 512],
                    start=(kc == 0), stop=(kc == NT - 1))
            dst = (q_sb if which == "q" else k_sb)[m][
                :, ncx * 512:(ncx + 1) * 512]
            boff = (0 if which == "q" else 2) + m
            if eng == "act":
                nc.scalar.activation(out=dst, in_=ps, func=AF.Identity,
                                     bias=qkvb[:, boff:boff + 1])
            else:
                nc.vector.tensor_scalar(out=dst, in0=ps,
                                        scalar1=qkvb[:, boff:boff + 1],
                                        scalar2=None, op0=ALU.add)

        def v_proj(nb, eng):
            ps = new_ops()
            for kc in range(NT):
                nc.tensor.matmul(
                    ps[:, 0:256], lhsT=X[kc][:, nb * 128:(nb + 1) * 128],
                    rhs=wT[kc][:, 4 * 128:6 * 128],
                    start=(kc == 0), stop=(kc == NT - 1))
            src = ps[:, 0:256].rearrange("p (h d) -> p h d", h=HL)
            dst = vT_sb[nb][:, :, 0:64]
            # v bias is folded at the opair copy; here plain convert
            if eng == "act":
                nc.scalar.activation(out=dst, in_=src, func=AF.Copy)
            else:
                nc.vector.tensor_copy(dst, src)
            nc.gpsimd.memset(vT_sb[nb][:, :, 64:65], 1.0)

        engs = ["act", "dve"]
        for m in range(MT):
            for ncx in range(4):
                kq_proj("k", m, ncx, engs[(m * 4 + ncx) % 2])
        for m in range(MT):
            for ncx in range(4):
                kq_proj("q", m, ncx, engs[(m * 4 + ncx + 1) % 2])
        for nb in range(NKB):
            v_proj(nb, engs[nb % 2])

        # ---- attention ----
        pairs = [(qc, m) for qc in range(QC) for m in range(MT)]
        opair = {}

        def emit_scores(qc, m, j):
            s = new_S()
            nc.tensor.matmul(
                s[:, 0:512], lhsT=k_sb[m][0:64, j * 128:(j + 1) * 128],
                rhs=q_sb[m][0:64, qc * 512:(qc + 1) * 512],
                start=True, stop=True, tile_position=(0, 0),
                skip_group_check=True)
            nc.tensor.matmul(
                s[:, 512:1024], lhsT=k_sb[m][64:128, j * 128:(j + 1) * 128],
                rhs=q_sb[m][64:128, qc * 512:(qc + 1) * 512],
                start=True, stop=True, tile_position=(64, 0),
                skip_group_check=True)
            return s

        def emit_exp(s, eT, j):
            if j in ACT_J:
                nc.scalar.activation(out=eT.bitcast(BF16), in_=s,
                                     func=AF.Exp, scale=SCALE)
            else:
                nc.vector.tensor_scalar(out=eT, in0=s,
                                        scalar1=A_S, scalar2=B_S,
                                        op0=ALU.mult, op1=ALU.add)

        def emit_av(av, m, j, eT):
            eb = eT.bitcast(BF16)
            for qb in range(4):
                for h in range(2):
                    nc.tensor.matmul(
                        av[:, qb * 2 + h, 0:65],
                        lhsT=eb[:, h * 512 + qb * 128:
                                h * 512 + (qb + 1) * 128],
                        rhs=vT_sb[j][:, 2 * m + h, 0:65],
                        start=(j == 0), stop=(j == NKB - 1))

        def drain_a(av):
            """recip + normalize into avn (4 ACT + 4 DVE)."""
            rden = drp.tile([128, 8, 1], F32, tag="rden", name="rden",
                            bufs=2)
            nc.vector.reciprocal(rden, av[:, :, 64:65])
            avn = drp.tile([128, 4, 128], BF16, tag="avn", name="avn",
                           bufs=2)
            for qb in range(4):
                for h in range(2):
                    src = av[:, qb * 2 + h, 0:64]
                    dst = avn[:, qb, h * 64:(h + 1) * 64]
                    if (qb + h) % 2 == 0:
                        nc.scalar.activation(out=dst, in_=src, func=AF.Copy,
                                             scale=rden[:, qb * 2 + h, :])
                    else:
                        nc.vector.tensor_scalar(
                            out=dst, in0=src,
                            scalar1=rden[:, qb * 2 + h, :],
                            scalar2=None, op0=ALU.mult)
            return avn

        def drain_b(qc, m, avn):
            """transpose + opair copy (+v bias)."""
            tps_flat = new_ops().bitcast(BF16)[:, 0:512]
            tps = tps_flat.rearrange("p (qb q) -> p qb q", qb=4)
            for qb in range(4):
                nc.tensor.transpose(tps[:, qb, :], avn[:, qb, :], ident)
            op = drp.tile([128, 512], BF16, tag=f"op{m}", name=f"op{m}",
                          bufs=2)
            nc.vector.tensor_scalar(out=op, in0=tps_flat,
                                    scalar1=qkvb[:, 4 + m:5 + m],
                                    scalar2=None, op0=ALU.add)
            opair[(qc, m)] = op

        def emit_outproj(qc, m2):
            ps = new_ops()
            for kc in range(MT):
                nc.tensor.matmul(
                    ps, lhsT=woutT[kc][:, m2 * 128:(m2 + 1) * 128],
                    rhs=opair[(qc, kc)],
                    start=(kc == 0), stop=(kc == MT - 1))
            yt = drp.tile([128, 512], F32, tag="yt", name="yt", bufs=4)
            if m2 % 2 == 0:
                nc.scalar.activation(out=yt, in_=ps, func=AF.Copy)
            else:
                nc.vector.tensor_copy(yt, ps)
            nc.sync.dma_start(
                out=y_out[m2 * 128:(m2 + 1) * 128,
                          qc * 512:(qc + 1) * 512],
                in_=yt)

        # One continuous software-pipelined stream over all pairs: at
        # stream slot i we emit scores+exp for stream[i] and the AV for
        # stream[i-2] (which may belong to the previous pair), so the
        # pair-boundary exp latency hides behind the next pair's scores.
        stream = [(qc, m, j) for qc, m in pairs for j in range(NKB)]
        avs = {}      # pair -> av psum tile
        eTs = {}      # (pair, j) -> eT tile
        avn_pend = None
        pending_op = None
        opj = {5: 0, 8: 1, 11: 2, 14: 3}
        for i, (qc, m, j) in enumerate(stream):
            s = emit_scores(qc, m, j)
            eT = expp.tile([128, 1024], I16, tag="eT", name="eT",
                           bufs=4)
            emit_exp(s, eT, j)
            eTs[(qc, m, j)] = eT
            if i >= 2:
                pqc, pm, pj = stream[i - 2]
                if (pqc, pm) not in avs:
                    avs[(pqc, pm)] = new_av()
                emit_av(avs[(pqc, pm)], pm, pj, eTs.pop((pqc, pm, pj)))
                if pj == NKB - 1:
                    # previous pair fully accumulated: drain it
                    avn_pend = (pqc, pm, drain_a(avs.pop((pqc, pm))))
            if j == 3 and avn_pend is not None:
                dqc, dm, avn = avn_pend
                drain_b(dqc, dm, avn)
                avn_pend = None
                if dm == 1:
                    pending_op = dqc
            if pending_op is not None and j in opj:
                emit_outproj(pending_op, opj[j])
                if j == 14:
                    pending_op = None
        # tail: last two AVs, drain last pair, final outproj
        for i in (len(stream) - 2, len(stream) - 1):
            pqc, pm, pj = stream[i]
            if (pqc, pm) not in avs:
                avs[(pqc, pm)] = new_av()
            emit_av(avs[(pqc, pm)], pm, pj, eTs.pop((pqc, pm, pj)))
        dqc, dm, avn = (stream[-1][0], stream[-1][1],
                        drain_a(avs.pop((stream[-1][0], stream[-1][1]))))
        drain_b(dqc, dm, avn)
        for m2 in range(NT):
            emit_outproj(QC - 1, m2)

    nc.compile()
    return nc


_NC = None


def _get_nc():
    global _NC
    if _NC is None:
        _NC = _build()
    return _NC


def _gblk():
    g = np.zeros((128, 8), dtype=np.float32)
    for p in range(128):
        g[p, p // CPG] = 1.0
    return g


def kernel(x, gn_gamma, gn_beta, w_qkv, w_out, b_out, trace=False):
    x = np.asarray(x, dtype=np.float32)
    w_qkv = np.asarray(w_qkv, np.float32)
    w_out = np.asarray(w_out, np.float32)
    gblk = _gblk()
    gbt = np.ascontiguousarray(gblk.T)
    gamma = np.asarray(gn_gamma, np.float32).reshape(C)
    beta = np.asarray(gn_beta, np.float32).reshape(C)
    gbo4 = np.zeros((128, 8), dtype=np.float32)
    for t in range(4):
        gbo4[:, 2 * t] = gamma[t * 128:(t + 1) * 128]
        gbo4[:, 2 * t + 1] = beta[t * 128:(t + 1) * 128]
    ident = np.eye(128, dtype=np.float32).astype(ml_dtypes.bfloat16)

    nc = _get_nc()
    in_maps = []
    for core in range(8):
        b, hg = core // 2, core % 2
        # wqkvT cols: [q m0, q m1, k m0, k m1, v m0, v m1] for local heads
        rows = np.concatenate([
            w_qkv[hg * 256:(hg + 1) * 256, :],
            w_qkv[C + hg * 256:C + (hg + 1) * 256, :],
            w_qkv[2 * C + hg * 256:2 * C + (hg + 1) * 256, :]], axis=0)
        wqkvT = np.ascontiguousarray(rows.T).astype(ml_dtypes.bfloat16)
        woutT = np.ascontiguousarray(
            w_out[:, hg * 256:(hg + 1) * 256].T).astype(ml_dtypes.bfloat16)
        in_maps.append({
            "x": np.ascontiguousarray(x[b]).astype(ml_dtypes.bfloat16),
            "wqkvT": wqkvT,
            "woutT": woutT,
            "gbo": gbo4,
            "gblk": gblk,
            "gbt": gbt,
            "ident": ident,
        })
    res = run_bass_kernel_spmd(nc, in_maps, core_ids=list(range(8)),
                               trace=trace)
    y = np.empty((B, C, N), dtype=np.float32)
    bo = np.asarray(b_out, np.float32).reshape(C, 1)
    for b in range(B):
        y[b] = (res.results[2 * b]["y"] + res.results[2 * b + 1]["y"]
                + x[b] + bo)
    if trace:
        kernel.last_results = res
    return y


# revision 13
# speedup vs baseline: 1.3543x; 1.0375x over previous
"""EnhancedTemporalAttention Trainium2 kernel (v2).

Full module: GroupNorm(32) -> QKV 1x1conv -> 8-head attention (softmax) ->
out 1x1conv + bias -> +residual, on x [4, 512, 2048] fp32.

Sharding: 8 cores = (batch b = core//2) x (head-half hg = core%2).  Each
core computes GroupNorm stats + its 4 heads' Q/K/V over the full sequence,
attention for all 2048 queries, and a partial out-projection (contraction
over its 256 channels).  Host sums the two partials per batch and adds
residual + b_out exactly in fp32.

GroupNorm is folded into the QKV weights: w' = w * scale_c (per input
channel, scaled in place on Pool), plus a K=1 matvec for the bias term
which rides the projection PSUM->SBUF copies as a per-partition bias.

Attention uses transposed scores (keys on partitions, [k,q] layout); exp
splits between ACT (exact, 19/32 per pair) and DVE (Schraudolph int16
bit-trick into bf16 bits, 13/32).  AV runs with eT stationary / vT moving
so each matmul is only 65 output rows; softmax denominators ride a ones
column on vT; normalization is a per-partition multiply in [q,d] layout,
then a PE transpose (bf16) back to [c,q] for the out-projection, whose
PSUM result DMAs straight to DRAM.
"""
import sys

sys.path.insert(0, "/opt/trn_rl_repo")

import numpy as np
import ml_dtypes

import concourse.bacc as bacc
import concourse.bass as bass
import concourse.tile as tile
from concourse import mybir
from concourse.bass_utils import run_bass_kernel_spmd

F32 = mybir.dt.float32
F32R = mybir.dt.float32r
BF16 = mybir.dt.bfloat16
I16 = mybir.dt.int16

B = 4
C = 512
N = 2048
H = 8
HL = 4             # local heads per core
D = 64
G = 32             # groupnorm groups
CPG = C // G       # 16 channels per group
EPS = 1e-4
SCALE = D ** -0.5
NT = C // 128      # 4 input-channel tiles
MT = 2             # local qkv channel tiles (256 local channels)
NKB = N // 128     # 16 key blocks
QC = 4             # query chunks of 512
AF = mybir.ActivationFunctionType
ALU = mybir.AluOpType

# Schraudolph exp into bf16 bits: i16 = s*A_S + B_S, bitcast -> bf16
A_S = 184.6650085 * SCALE
B_S = 16249.1
# j's whose (whole-j, both-head) exp runs on ACT exactly; the rest run
# the Schraudolph bit-trick on DVE.
ACT_J = (0, 1, 3, 5, 7, 9, 11, 13, 15)


def _build(taps=False):
    nc = bacc.Bacc("TRN2", target_bir_lowering=False, debug=False)
    x_in = nc.dram_tensor("x", [C, N], BF16, kind="ExternalInput").ap()
    wqkvT_in = nc.dram_tensor("wqkvT", [C, 6 * 128], BF16,
                              kind="ExternalInput").ap()
    woutT_in = nc.dram_tensor("woutT", [MT * 128, C], BF16,
                              kind="ExternalInput").ap()
    gbo_in = nc.dram_tensor("gbo", [128, 8], F32, kind="ExternalInput").ap()
    gblk_in = nc.dram_tensor("gblk", [128, 8], F32, kind="ExternalInput").ap()
    gbt_in = nc.dram_tensor("gbt", [8, 128], F32, kind="ExternalInput").ap()
    id_in = nc.dram_tensor("ident", [128, 128], BF16,
                           kind="ExternalInput").ap()
    y_out = nc.dram_tensor("y", [C, N], F32, kind="ExternalOutput").ap()

    from contextlib import ExitStack
    with tile.TileContext(nc) as tc, ExitStack() as ctx:
        persist = ctx.enter_context(tc.tile_pool(name="persist", bufs=1))
        gn = ctx.enter_context(tc.tile_pool(name="gn", bufs=1))
        pspool = ctx.enter_context(tc.tile_pool(name="ps", bufs=1,
                                                space="PSUM"))
        expp = ctx.enter_context(tc.tile_pool(name="expp", bufs=1))
        drp = ctx.enter_context(tc.tile_pool(name="drp", bufs=1))

        # ---- persistent tiles ----
        X = [persist.tile([128, N], BF16, tag=f"X{t}", name=f"X{t}")
             for t in range(NT)]
        wT = [persist.tile([128, 6 * 128], BF16, tag=f"wT{kc}",
                           name=f"wT{kc}") for kc in range(NT)]
        woutT = [persist.tile([128, C], BF16, tag=f"woT{m}", name=f"woT{m}")
                 for m in range(MT)]
        q_sb = [persist.tile([128, N], BF16, tag=f"q{m}", name=f"q{m}")
                for m in range(MT)]
        k_sb = [persist.tile([128, N], BF16, tag=f"k{m}", name=f"k{m}")
                for m in range(MT)]
        vT_sb = [persist.tile([128, HL, 66], BF16, tag=f"vT{nb}",
                              name=f"vT{nb}") for nb in range(NKB)]
        ident = persist.tile([128, 128], BF16, tag="ident", name="ident")

        # PSUM rings: S 3x4KB + av 4KB = 16KB exactly; every other
        # psum user (GN, projections, transpose, out-proj) shares the
        # S ring, using a [:, 0:512] half-slot view.
        def new_S():
            return pspool.tile([128, 1024], F32, tag="S", name="S", bufs=3)

        def new_ops():
            return new_S()[:, 0:512]

        def new_av():
            return pspool.tile([128, 8, 128], F32, tag="av", name="av",
                               bufs=1)

        # ---- input loads ----
        # x: 8 chunks of [128,1024] on the HWDGE path (critical for stats);
        # misc/gbt woven in after chunk 3.  Weights + ident ride the
        # software DGE (gpsimd) so they bypass the serialized HWDGE device.
        gbo4 = gn.tile([128, 8], F32, tag="gbo4")
        gblk = gn.tile([128, 8], F32R, tag="gblk")
        gbt = gn.tile([8, 128], F32R, tag="gbt")
        for t in range(NT):
            for half in range(2):
                nc.sync.dma_start(
                    out=X[t][:, half * 1024:(half + 1) * 1024],
                    in_=x_in[t * 128:(t + 1) * 128,
                             half * 1024:(half + 1) * 1024])
        nc.sync.dma_start(out=gbo4, in_=gbo_in)
        nc.sync.dma_start(out=gblk, in_=gblk_in.bitcast(F32R))
        nc.sync.dma_start(out=gbt, in_=gbt_in.bitcast(F32R))
        for kc in range(NT):
            nc.sync.dma_start(out=wT[kc],
                              in_=wqkvT_in[kc * 128:(kc + 1) * 128, :])
        for m in range(MT):
            nc.sync.dma_start(out=woutT[m],
                              in_=woutT_in[m * 128:(m + 1) * 128, :])
        nc.sync.dma_start(out=ident, in_=id_in)
        gbo = [gbo4[:, 2 * t:2 * t + 2] for t in range(NT)]

        # ---- GroupNorm stats ----
        eps_t = gn.tile([G, 1], F32, tag="eps_t")
        nc.vector.memset(eps_t, EPS)
        sqw = gn.tile([G, 1], F32, tag="sqw")
        nc.scalar.activation(out=sqw, in_=eps_t, func=AF.Sqrt)
        mvv = []
        for t in range(NT):
            stats = gn.tile([128, 4, 6], F32, tag=f"st{t}", name=f"st{t}")
            for sg in range(4):
                nc.vector.bn_stats(out=stats[:, sg, :],
                                   in_=X[t][:, sg * 512:(sg + 1) * 512])
            mv = gn.tile([128, 2], F32, tag=f"mv{t}", name=f"mv{t}")
            nc.vector.bn_aggr(out=mv, in_=stats)
            mt = gn.tile([128, 2], F32R, tag=f"mvv{t}", name=f"mvv{t}")
            nc.vector.tensor_copy(mt[:, 0:1], mv[:, 0:1])
            nc.vector.scalar_tensor_tensor(
                out=mt[:, 1:2], in0=mv[:, 0:1], scalar=mv[:, 0:1],
                in1=mv[:, 1:2], op0=ALU.mult, op1=ALU.add)
            mvv.append(mt)
        g8ps = new_ops()
        for t in range(NT):
            nc.tensor.matmul(g8ps[0:8, t * 2:(t + 1) * 2],
                             lhsT=gblk, rhs=mvv[t],
                             start=(t == 0), stop=(t == NT - 1),
                             skip_group_check=True)
        g8 = gn.tile([8, NT, 2], F32, tag="g8")
        nc.vector.tensor_copy(g8.rearrange("p t s -> p (t s)"),
                              g8ps[0:8, 0:8])
        mean8 = gn.tile([8, NT], F32, tag="mean8")
        nc.vector.tensor_scalar_mul(mean8, g8[:, :, 0], 1.0 / CPG)
        ex28 = gn.tile([8, NT], F32, tag="ex28")
        nc.vector.tensor_scalar_mul(ex28, g8[:, :, 1], 1.0 / CPG)
        msq8 = gn.tile([8, NT], F32, tag="msq8")
        nc.vector.tensor_mul(msq8, mean8, mean8)
        var8 = gn.tile([8, NT], F32, tag="var8")
        nc.vector.tensor_tensor(out=var8, in0=ex28, in1=msq8,
                                op=ALU.subtract)
        std8 = gn.tile([8, NT], F32, tag="std8")
        nc.scalar.activation(out=std8, in_=var8, func=AF.Sqrt,
                             bias=eps_t[0:8, :])
        rstd8 = gn.tile([8, NT], F32, tag="rstd8")
        nc.vector.reciprocal(rstd8, std8)
        # preload the Exp table; chained after the real Sqrt via std8
        warm = gn.tile([8, NT], F32, tag="warm")
        nc.scalar.activation(out=warm, in_=std8, func=AF.Exp)
        mr8 = gn.tile([8, NT, 2], F32R, tag="mr8")
        nc.vector.tensor_copy(mr8[:, :, 0:1],
                              mean8.rearrange("p (t o) -> p t o", o=1))
        nc.vector.tensor_copy(mr8[:, :, 1:2],
                              rstd8.rearrange("p (t o) -> p t o", o=1))
        msps = new_ops()
        for t in range(NT):
            nc.tensor.matmul(msps[:, t * 2:(t + 1) * 2],
                             lhsT=gbt, rhs=mr8[:, t, :],
                             start=(t == 0), stop=(t == NT - 1),
                             skip_group_check=True)
        mscall = msps[:, 0:2 * NT].rearrange("p (t s) -> p t s", s=2)

        # per-channel scale_c = rstd*gamma, bias_c = beta - mean*scale_c
        qkvb_ps = None
        scale_c = []
        for t in range(NT):
            sc = gn.tile([128, 1], F32, tag=f"sc{t}", name=f"sc{t}")
            nc.vector.tensor_mul(sc, mscall[:, t, 1:2], gbo[t][:, 0:1])
            scale_c.append(sc)
            tmp = gn.tile([128, 1], F32, tag=f"tmp{t}", name=f"tmp{t}")
            nc.vector.tensor_mul(tmp, mscall[:, t, 0:1], sc)
            bias_c = gn.tile([128, 1], F32, tag=f"bc{t}", name=f"bc{t}")
            nc.vector.tensor_tensor(out=bias_c, in0=gbo[t][:, 1:2],
                                    in1=tmp, op=ALU.subtract)
            bb = gn.tile([128, 1], BF16, tag=f"bb{t}", name=f"bb{t}")
            nc.vector.tensor_copy(bb, bias_c)
            # qkv bias matvec against RAW weights (before scaling)
            if qkvb_ps is None:
                qkvb_ps = new_ops()
            for oc in range(6):
                nc.tensor.matmul(qkvb_ps[:, 6 * t + oc:6 * t + oc + 1],
                                 lhsT=wT[t][:, oc * 128:(oc + 1) * 128],
                                 rhs=bb, start=True, stop=True,
                                 skip_group_check=True)
        # reduce the 4 kc partials: qkvb[:, oc] = sum_t qkvb_ps[:, 6t+oc]
        qkvb_all = gn.tile([128, NT, 6], F32, tag="qkvb_all")
        nc.vector.tensor_copy(qkvb_all.rearrange("p t s -> p (t s)"),
                              qkvb_ps[:, 0:24])
        qkvb01 = gn.tile([128, 6], F32, tag="qkvb01")
        nc.vector.tensor_tensor(out=qkvb01, in0=qkvb_all[:, 0, :],
                                in1=qkvb_all[:, 1, :], op=ALU.add)
        qkvb23 = gn.tile([128, 6], F32, tag="qkvb23")
        nc.vector.tensor_tensor(out=qkvb23, in0=qkvb_all[:, 2, :],
                                in1=qkvb_all[:, 3, :], op=ALU.add)
        qkvb = gn.tile([128, 6], F32, tag="qkvb")
        nc.vector.tensor_tensor(out=qkvb, in0=qkvb01, in1=qkvb23,
                                op=ALU.add)
        # scale weights in place (per input-channel partition)
        for t in range(NT):
            if t < 2:
                nc.scalar.activation(out=wT[t], in_=wT[t], func=AF.Copy,
                                     scale=scale_c[t])
            else:
                nc.gpsimd.tensor_scalar(out=wT[t], in0=wT[t],
                                        scalar1=scale_c[t], scalar2=None,
                                        op0=ALU.mult)

        # ---- projections ----
        def kq_proj(which, m, ncx, eng):
            ps = new_ops()
            col0 = (m if which == "q" else 2 + m) * 128
            for kc in range(NT):
                nc.tensor.matmul(
                    ps, lhsT=wT[kc][:, col0:col0 + 128],
                    rhs=X[kc][:, ncx * 512:(ncx + 1) * 512],
                    start=(kc == 0), stop=(kc == NT - 1))
            dst = (q_sb if which == "q" else k_sb)[m][
                :, ncx * 512:(ncx + 1) * 512]
            boff = (0 if which == "q" else 2) + m
            if eng == "act":
                nc.scalar.activation(out=dst, in_=ps, func=AF.Identity,
                                     bias=qkvb[:, boff:boff + 1])
            else:
                nc.vector.tensor_scalar(out=dst, in0=ps,
                                        scalar1=qkvb[:, boff:boff + 1],
                                        scalar2=None, op0=ALU.add)

        def v_proj(nb, eng):
            ps = new_ops()
            for kc in range(NT):
                nc.tensor.matmul(
                    ps[:, 0:256], lhsT=X[kc][:, nb * 128:(nb + 1) * 128],
                    rhs=wT[kc][:, 4 * 128:6 * 128],
                    start=(kc == 0), stop=(kc == NT - 1))
            src = ps[:, 0:256].rearrange("p (h d) -> p h d", h=HL)
            dst = vT_sb[nb][:, :, 0:64]
            # v bias is folded at the opair copy; here plain convert
            if eng == "act":
                nc.scalar.activation(out=dst, in_=src, func=AF.Copy)
            else:
                nc.vector.tensor_copy(dst, src)
            nc.gpsimd.memset(vT_sb[nb][:, :, 64:65], 1.0)

        engs = ["act", "dve"]
        for m in range(MT):
            for ncx in range(4):
                kq_proj("k", m, ncx, engs[(m * 4 + ncx) % 2])
        for m in range(MT):
            for ncx in range(4):
                kq_proj("q", m, ncx, engs[(m * 4 + ncx + 1) % 2])
        for nb in range(NKB):
            v_proj(nb, engs[nb % 2])

        # ---- attention ----
        pairs = [(qc, m) for qc in range(QC) for m in range(MT)]
        opair = {}

        def emit_scores(qc, m, j):
            s = new_S()
            nc.tensor.matmul(
                s[:, 0:512], lhsT=k_sb[m][0:64, j * 128:(j + 1) * 128],
                rhs=q_sb[m][0:64, qc * 512:(qc + 1) * 512],
                start=True, stop=True, tile_position=(0, 0),
                skip_group_check=True)
            nc.tensor.matmul(
                s[:, 512:1024], lhsT=k_sb[m][64:128, j * 128:(j + 1) * 128],
                rhs=q_sb[m][64:128, qc * 512:(qc + 1) * 512],
                start=True, stop=True, tile_position=(64, 0),
                skip_group_check=True)
            return s

        def emit_exp(s, eT, j):
            if j in ACT_J:
                nc.scalar.activation(out=eT.bitcast(BF16), in_=s,
                                     func=AF.Exp, scale=SCALE)
            else:
                nc.vector.tensor_scalar(out=eT, in0=s,
                                        scalar1=A_S, scalar2=B_S,
                                        op0=ALU.mult, op1=ALU.add)

        def emit_av(av, m, j, eT):
            eb = eT.bitcast(BF16)
            for qb in range(4):
                for h in range(2):
                    nc.tensor.matmul(
                        av[:, qb * 2 + h, 0:65],
                        lhsT=eb[:, h * 512 + qb * 128:
                                h * 512 + (qb + 1) * 128],
                        rhs=vT_sb[j][:, 2 * m + h, 0:65],
                        start=(j == 0), stop=(j == NKB - 1))

        def drain_a(av):
            """recip + normalize into avn.  DVE is free of exps at the
            pair boundary (j0/j1 go to ACT), so recip + the qb 0/1 norms
            run immediately on DVE; qb 2/3 norms queue on ACT after its
            j0/j1 exps.  drain_b transposes are split to match."""
            rden = drp.tile([128, 8, 1], F32, tag="rden", name="rden",
                            bufs=2)
            nc.vector.reciprocal(rden, av[:, :, 64:65])
            avn = drp.tile([128, 4, 128], BF16, tag="avn", name="avn",
                           bufs=2)
            for qb in range(4):
                eng = nc.vector if qb < 2 else nc.scalar
                for h in range(2):
                    src = av[:, qb * 2 + h, 0:64]
                    dst = avn[:, qb, h * 64:(h + 1) * 64]
                    if qb < 2:
                        nc.vector.tensor_scalar(
                            out=dst, in0=src,
                            scalar1=rden[:, qb * 2 + h, :],
                            scalar2=None, op0=ALU.mult)
                    else:
                        nc.scalar.activation(out=dst, in_=src, func=AF.Copy,
                                             scale=rden[:, qb * 2 + h, :])
            return avn

        def drain_b(qc, m, avn, halves=(0, 1)):
            """transpose + opair copy (+v bias), split by qb halves."""
            if 0 in halves:
                tps_flat = new_ops().bitcast(BF16)[:, 0:512]
                op = drp.tile([128, 512], BF16, tag=f"op{m}",
                              name=f"op{m}", bufs=2)
                drain_b.state = (tps_flat, op)
            tps_flat, op = drain_b.state
            tps = tps_flat.rearrange("p (qb q) -> p qb q", qb=4)
            for half in halves:
                for qb in (2 * half, 2 * half + 1):
                    nc.tensor.transpose(tps[:, qb, :], avn[:, qb, :], ident)
                nc.vector.tensor_scalar(
                    out=op[:, half * 256:(half + 1) * 256],
                    in0=tps_flat[:, half * 256:(half + 1) * 256],
                    scalar1=qkvb[:, 4 + m:5 + m],
                    scalar2=None, op0=ALU.add)
            opair[(qc, m)] = op

        def emit_outproj(qc, m2):
            ps = new_ops()
            for kc in range(MT):
                nc.tensor.matmul(
                    ps, lhsT=woutT[kc][:, m2 * 128:(m2 + 1) * 128],
                    rhs=opair[(qc, kc)],
                    start=(kc == 0), stop=(kc == MT - 1))
            yt = drp.tile([128, 512], F32, tag="yt", name="yt", bufs=4)
            if m2 % 2 == 0:
                nc.scalar.activation(out=yt, in_=ps, func=AF.Copy)
            else:
                nc.vector.tensor_copy(yt, ps)
            nc.sync.dma_start(
                out=y_out[m2 * 128:(m2 + 1) * 128,
                          qc * 512:(qc + 1) * 512],
                in_=yt)

        # One continuous software-pipelined stream over all pairs: at
        # stream slot i we emit scores+exp for stream[i] and the AV for
        # stream[i-2] (which may belong to the previous pair), so the
        # pair-boundary exp latency hides behind the next pair's scores.
        stream = [(qc, m, j) for qc, m in pairs for j in range(NKB)]
        avs = {}      # pair -> av psum tile
        eTs = {}      # (pair, j) -> eT tile
        avn_pend = None
        pending_op = None
        opj = {6: 0, 8: 1, 10: 2, 12: 3}
        for i, (qc, m, j) in enumerate(stream):
            s = emit_scores(qc, m, j)
            eT = expp.tile([128, 1024], I16, tag="eT", name="eT",
                           bufs=4)
            emit_exp(s, eT, j)
            eTs[(qc, m, j)] = eT
            if i >= 2:
                pqc, pm, pj = stream[i - 2]
                if (pqc, pm) not in avs:
                    avs[(pqc, pm)] = new_av()
                emit_av(avs[(pqc, pm)], pm, pj, eTs.pop((pqc, pm, pj)))
                if pj == NKB - 1:
                    # previous pair fully accumulated: drain it
                    avn_pend = (pqc, pm, drain_a(avs.pop((pqc, pm))))
            if j == 3 and avn_pend is not None:
                drain_b(avn_pend[0], avn_pend[1], avn_pend[2], halves=(0,))
            if j == 4 and avn_pend is not None:
                dqc, dm, avn = avn_pend
                drain_b(dqc, dm, avn, halves=(1,))
                avn_pend = None
                if dm == 1:
                    pending_op = dqc
            if pending_op is not None and j in opj:
                emit_outproj(pending_op, opj[j])
                if j == max(opj):
                    pending_op = None
        # tail: last two AVs, drain last pair, final outproj
        for i in (len(stream) - 2, len(stream) - 1):
            pqc, pm, pj = stream[i]
            if (pqc, pm) not in avs:
                avs[(pqc, pm)] = new_av()
            emit_av(avs[(pqc, pm)], pm, pj, eTs.pop((pqc, pm, pj)))
        dqc, dm, avn = (stream[-1][0], stream[-1][1],
                        drain_a(avs.pop((stream[-1][0], stream[-1][1]))))
        drain_b(dqc, dm, avn)
        for m2 in range(NT):
            emit_outproj(QC - 1, m2)

    nc.compile()
    return nc


_NC = None


def _get_nc():
    global _NC
    if _NC is None:
        _NC = _build()
    return _NC


def _gblk():
    g = np.zeros((128, 8), dtype=np.float32)
    for p in range(128):
        g[p, p // CPG] = 1.0
    return g


def kernel(x, gn_gamma, gn_beta, w_qkv, w_out, b_out, trace=False):
    x = np.asarray(x, dtype=np.float32)
    w_qkv = np.asarray(w_qkv, np.float32)
    w_out = np.asarray(w_out, np.float32)
    gblk = _gblk()
    gbt = np.ascontiguousarray(gblk.T)
    gamma = np.asarray(gn_gamma, np.float32).reshape(C)
    beta = np.asarray(gn_beta, np.float32).reshape(C)
    gbo4 = np.zeros((128, 8), dtype=np.float32)
    for t in range(4):
        gbo4[:, 2 * t] = gamma[t * 128:(t + 1) * 128]
        gbo4[:, 2 * t + 1] = beta[t * 128:(t + 1) * 128]
    ident = np.eye(128, dtype=np.float32).astype(ml_dtypes.bfloat16)

    nc = _get_nc()
    in_maps = []
    for core in range(8):
        b, hg = core // 2, core % 2
        # wqkvT cols: [q m0, q m1, k m0, k m1, v m0, v m1] for local heads
        rows = np.concatenate([
            w_qkv[hg * 256:(hg + 1) * 256, :],
            w_qkv[C + hg * 256:C + (hg + 1) * 256, :],
            w_qkv[2 * C + hg * 256:2 * C + (hg + 1) * 256, :]], axis=0)
        wqkvT = np.ascontiguousarray(rows.T).astype(ml_dtypes.bfloat16)
        woutT = np.ascontiguousarray(
            w_out[:, hg * 256:(hg + 1) * 256].T).astype(ml_dtypes.bfloat16)
        in_maps.append({
            "x": np.ascontiguousarray(x[b]).astype(ml_dtypes.bfloat16),
            "wqkvT": wqkvT,
            "woutT": woutT,
            "gbo": gbo4,
            "gblk": gblk,
            "gbt": gbt,
            "ident": ident,
        })
    res = run_bass_kernel_spmd(nc, in_maps, core_ids=list(range(8)),
                               trace=trace)
    y = np.empty((B, C, N), dtype=np.float32)
    bo = np.asarray(b_out, np.float32).reshape(C, 1)
    for b in range(B):
        y[b] = (res.results[2 * b]["y"] + res.results[2 * b + 1]["y"]
                + x[b] + bo)
    if trace:
        kernel.last_results = res
    return y
